# revision 23
# baseline (speedup 1.0000x reference)
"""3x3 median filter (zero-padded) on TRN2, 8 NeuronCores, bf16 datapath.

Input  x: (32, 3, 512, 512) float32
Output  : (32, 3, 512, 512) float32.

Accuracy: the median network only ever SELECTS one of its 9 inputs (min/max
ops create no new values), so the device-side bf16 result equals the bf16
rounding of the element that is the median of the rounded window. Order
statistics are 1-Lipschitz under sup-norm perturbation, so end-to-end error
is <= 2^-8 relative -- far inside the 2e-2 gate. Measured 3.4e-3.

Strategy
--------
Pure data parallel: batch dim sharded 4-per-core across 8 cores; per core
12 images (4 batch x 3 chan) in 2 groups of 6 images x 2 vertical halves.

bf16 doubles DVE tensor_tensor throughput (2x_1P perf mode) but ONLY for
unit-stride 4-byte-aligned access patterns, so the horizontal stage is
restructured from the fp32 baseline's stride-2 parity tricks into dense
shifted-field form. Per field F in {MN,MD,MX}: s1F[j]=F[j+1] is the ONLY
odd-element shift (a ScalarE copy -- ACT is otherwise idle), then
   P[j] = op(F[j], s1F[j])          # aligned TT, 2x
   R[j] = op(P[j], F[j+2])          # +2 elems = 4B-aligned shift, 2x
R[j] = sliding-3 result centered at col j+1; the final med3(Rmn,Rmd,Rmx)
writes into an OUT grid whose per-image segment holds col c at position
c+1, so the wide write starts at even offset 2 and the DMA store (which
doesn't care about alignment) un-shifts.

Both row parities' fields live in ONE fused [128, 12*514] tile (odd-parity
images = segments 0..5, even = 6..11), so stage 2 is 12 double-width ops
per block instead of 24 -- per-op overhead (58-cycle issue + ~90ns DRAIN)
is the only thing that changes, the streamed cycles are identical. Output
cols 0 and 511 (windows containing the zero pad column) are 7 tiny ops on
gathered grid positions {0,510} across all 12 segments.

Grid: per-image segment width 514 (even -> every segment start keeps 4B
parity). Segment positions 512..513 are scratch: stage-1 ops run flat over
the whole grid and compute garbage there; no stored output reads a garbage
lane (out cols 0/511 come from the boundary path).

SBUF fits via aggressive aliasing with DVE-program-order-safe lifetimes:
stage-1's qmn/qmx/t_o/t_e live in halves of stage-2's Pmx/Qmx slots;
stage-2's Rmx/tmd/Rmd overwrite the dead MN/MX/MD field buffers; MN1/MX1/TF
overwrite Pmn/Qmn/Qmx. Stage-1 emits fields in MN,MX,MD order and ACT
copies s1MN,s1MX,s1MD in that order so every copy lands before the DVE
needs it, with no stall.

Image rows 0 and 511 (windows contain the zero pad row): one small
24-partition pass issued LAST so it fills the DVE-idle tail while the
final block's output stores (sent to the idle HWDGE queues) drain. Its
tiny loads are issued up front. Block 0's loads+stage-1 ramp up in 1+2+3
image chunks sized to the ~250GB/s strided-row-gather DMA rate, so the
DVE starts ~2us after the first 0.25MB lands instead of waiting for 3MB.

Engine budget per core: DVE 17 TT/elem at 2x ~= 236us busy (the floor for
this decomposition), ACT ~75us, DMA ~19MB. Loads on the SP+ACT HWDGE
queues, mid-kernel stores on the GpSimd SWDGE queue.
"""
import sys

if "/opt/trn_rl_repo" not in sys.path:
    sys.path.insert(0, "/opt/trn_rl_repo")

import numpy as np
import ml_dtypes
import concourse.bacc as bacc
import concourse.mybir as mybir
import concourse.tile as tile
from concourse import bass_utils

B, C, H, W = 32, 3, 512, 512
N_CORES = 8
B_PER = B // N_CORES          # 4 batches per core
NIMG = B_PER * C              # 12 images per core
GIMG = 6                      # images per tile group
PW = W + 2                    # per-image grid width (514, even)
FP = GIMG * PW                # half (one parity) grid width (3084)
FP2 = 2 * FP                  # fused two-parity grid width (6168)
NSEG = 2 * GIMG               # segments in a fused tile (12)
HH = H // 2                   # 256 rows per vertical half
P = 128                       # partitions = row pairs per half
NE = 2 * NIMG                 # partitions in the edge-rows pass (24)

BF16 = mybir.dt.bfloat16
MIN = mybir.AluOpType.min
MAX = mybir.AluOpType.max

_PROGRAM = None


def _seg(T, npart, nseg):
    """[npart, nseg, 514] per-image-segment view."""
    return T[:].rearrange("p (i w) -> p i w", w=PW)[0:npart, 0:nseg]


def _stage2_fused(nc, pm, MN, MD, MX, OUT):
    """Both parities of stage 2 in double-width ops on the fused grids.
    ScalarE does the three s1 shifts (issued in MN,MX,MD order to match
    the DVE's consumption order). Aliases overwrite only dead buffers."""
    NI = FP2 - 2
    fl = lambda T, a, b: T[:][:, a:b]

    s1 = {}
    for nm, F in (("MN", MN), ("MX", MX), ("MD", MD)):
        T = pm.tile([P, FP2], BF16, tag=f"s1{nm}", name=f"s1{nm}")
        nc.scalar.copy(T[:][:, 0 : FP2 - 1], F[:][:, 1:FP2])
        s1[nm] = T

    def t2(tag):
        return pm.tile([P, FP2], BF16, tag=tag, name=tag)

    def alias(tag, name):
        return pm.tile([P, FP2], BF16, tag=tag, name=name)

    Pmn, Pmx, Qmn, Qmx, Rmn = t2("Pmn"), t2("Pmx"), t2("Qmn"), t2("Qmx"), t2("Rmn")
    Rmx = alias("fMN", "Rmx")   # MN field dead after Rmn
    tmd = alias("fMX", "tmd")   # MX field dead after Rmx
    Rmd = alias("fMD", "Rmd")   # MD field dead after tmd
    MN1 = alias("Pmn", "MN1")
    MX1 = alias("Qmn", "MX1")
    TF = alias("Qmx", "TF")

    tt = nc.vector.tensor_tensor
    gv = lambda T: _seg(T, P, NSEG)[:, :, 0:511:510]
    bt = lambda tag: pm.tile([P, 2 * NSEG], BF16, tag=f"{tag}b", name=f"{tag}b")
    bv = lambda T: T[:].rearrange("p (i c) -> p i c", c=2)[0:P, 0:NSEG]
    BA, BB, BC = bt("BA"), bt("BB"), bt("BC")
    B1, B2, B3 = bt("B1"), bt("B2"), bt("B3")

    tt(fl(Pmn, 0, NI), fl(MN, 0, NI), fl(s1["MN"], 0, NI), op=MAX)
    tt(fl(Pmx, 0, NI), fl(MX, 0, NI), fl(s1["MX"], 0, NI), op=MIN)
    nc.vector.tensor_scalar_max(bv(BA), gv(Pmn), 0.0)
    nc.vector.tensor_scalar_min(bv(BC), gv(Pmx), 0.0)
    tt(fl(Rmn, 0, NI), fl(Pmn, 0, NI), fl(MN, 2, FP2), op=MAX)
    tt(fl(Rmx, 0, NI), fl(Pmx, 0, NI), fl(MX, 2, FP2), op=MIN)
    tt(fl(Qmn, 0, NI), fl(MD, 0, NI), fl(s1["MD"], 0, NI), op=MIN)
    tt(fl(Qmx, 0, NI), fl(MD, 0, NI), fl(s1["MD"], 0, NI), op=MAX)
    nc.vector.scalar_tensor_tensor(bv(BB), gv(Qmx), 0.0, gv(Qmn), op0=MIN, op1=MAX)
    tt(fl(tmd, 0, NI), fl(Qmx, 0, NI), fl(MD, 2, FP2), op=MIN)
    tt(fl(Rmd, 0, NI), fl(Qmn, 0, NI), fl(tmd, 0, NI), op=MAX)
    # med3(Rmn, Rmd, Rmx); last wide op writes output cols 1..510 at grid
    # positions 2..511 (even start -> stays in 2x mode)
    tt(fl(MN1, 0, NI), fl(Rmn, 0, NI), fl(Rmd, 0, NI), op=MIN)
    tt(fl(MX1, 0, NI), fl(Rmn, 0, NI), fl(Rmd, 0, NI), op=MAX)
    tt(fl(TF, 0, NI), fl(MX1, 0, NI), fl(Rmx, 0, NI), op=MIN)
    ov = _seg(OUT, P, NSEG)[:, :, 2:512]
    tt(ov, _seg(MN1, P, NSEG)[:, :, 0:510], _seg(TF, P, NSEG)[:, :, 0:510],
       op=MAX)
    # boundary med3 -> OUT positions {1, 512}
    tt(bv(B1), bv(BA), bv(BB), op=MIN)
    tt(bv(B2), bv(BA), bv(BB), op=MAX)
    tt(bv(B3), bv(B2), bv(BC), op=MIN)
    tt(_seg(OUT, P, NSEG)[:, :, 1:513:511], bv(B1), bv(B3), op=MAX)


def _block(nc, pio, pm, xh, oh, g, half, first=False, last=False):
    """One vertical half of one image group: odd output rows r0+1..r0+255,
    even rows r0+2..r0+256 (halves overlap by two rows so every DMA is a
    full 128-partition transfer). Rows 0 and 511 via the edge pass.
    first=True: loads and stage 1 ramp in 1+2+3 image chunks (cold start).
    last=True: stores go to the by-then-idle HWDGE queues so the final
    drain overlaps the edge pass."""
    r0 = 0 if half == 0 else H - HH - 2
    i0 = GIMG * g

    E = pio.tile([P, FP], BF16, tag="E", name="E")
    O = pio.tile([P, FP], BF16, tag="O", name="O")
    E_sh = pio.tile([P, FP], BF16, tag="E_sh", name="E_sh")
    O_sh2 = pio.tile([P, FP], BF16, tag="O_sh2", name="O_sh2")

    # scratch cols 512..513 of each segment are read by the flat stage-1
    # ops but never loaded: define them so no lane is uninitialized
    for T in (E, O, E_sh, O_sh2):
        nc.gpsimd.memset(_seg(T, P, GIMG)[:, :, W:PW], 0.0)

    def loads(ia, ib):
        lv = lambda T: _seg(T, P, GIMG)[:, ia:ib, 0:W]
        im = lambda r_lo: xh[
            r_lo : min(r_lo + 2 * P, H) : 2, i0 + ia : i0 + ib, :
        ]
        # queue order matters (HWDGE queues are FIFOs): the (O, E_sh) pair
        # feeds the first op of the block, so those loads go first
        nc.sync.dma_start(lv(E_sh), im(r0 + 2))     # rows r0+2p+2
        nc.scalar.dma_start(lv(O), im(r0 + 1))      # rows r0+2p+1
        nc.sync.dma_start(lv(E), im(r0))            # rows r0+2p
        nc.scalar.dma_start(lv(O_sh2), im(r0 + 3))  # rows r0+2p+3

    # stage-1 temps live in halves of stage-2 slots that are written later
    qpair = pm.tile([P, FP2], BF16, tag="Pmx", name="qpair")
    tpair = pm.tile([P, FP2], BF16, tag="Qmx", name="tpair")
    MN2 = pm.tile([P, FP2], BF16, tag="fMN", name="fMN")
    MD2 = pm.tile([P, FP2], BF16, tag="fMD", name="fMD")
    MX2 = pm.tile([P, FP2], BF16, tag="fMX", name="fMX")

    tt = nc.vector.tensor_tensor

    def stage1(ia, ib):
        a, b = ia * PW, ib * PW
        qv = lambda h: qpair[:][:, h * FP + a : h * FP + b]
        tv = lambda h: tpair[:][:, h * FP + a : h * FP + b]
        f = lambda T, h: T[:][:, h * FP + a : h * FP + b]
        sv = lambda T: T[:][:, a:b]
        qmn, qmx = qv(0), qv(1)
        # shared pair = (O, E_sh) = rows (2p+1, 2p+2)
        tt(qmn, sv(O), sv(E_sh), op=MIN)
        tt(qmx, sv(O), sv(E_sh), op=MAX)
        # field completion order MN, MX, MD matches the ACT copy order in
        # _stage2_fused so no s1 copy ever stalls the DVE.
        # odd output rows r0+2p+1: pair + E; even rows: pair + O_sh2
        tt(f(MN2, 0), qmn, sv(E), op=MIN)
        tt(f(MN2, 1), qmn, sv(O_sh2), op=MIN)
        tt(f(MX2, 0), qmx, sv(E), op=MAX)
        tt(f(MX2, 1), qmx, sv(O_sh2), op=MAX)
        tt(tv(0), qmx, sv(E), op=MIN)
        tt(f(MD2, 0), qmn, tv(0), op=MAX)
        tt(tv(1), qmx, sv(O_sh2), op=MIN)
        tt(f(MD2, 1), qmn, tv(1), op=MAX)

    if first:
        for ia, ib in ((0, 1), (1, 3), (3, 6)):
            loads(ia, ib)
            stage1(ia, ib)
    else:
        loads(0, GIMG)
        stage1(0, GIMG)

    OUT = pio.tile([P, FP2], BF16, tag="OUT", name="OUT")
    _stage2_fused(nc, pm, MN2, MD2, MX2, OUT)

    out_img = lambda r_lo: oh[r_lo : min(r_lo + 2 * P, H) : 2, i0 : i0 + GIMG, :]
    osv = _seg(OUT, P, NSEG)
    if last:
        # HWDGE queues are idle by now (all loads issued); their stores
        # drain while the edge pass runs, shrinking the end-of-kernel tail
        nc.sync.dma_start(out_img(r0 + 1), osv[:, 0:GIMG, 1:513])
        nc.scalar.dma_start(out_img(r0 + 2), osv[:, GIMG:NSEG, 1:513])
    else:
        # stores on the SWDGE queue so they never block later blocks' loads
        nc.gpsimd.dma_start(out_img(r0 + 1), osv[:, 0:GIMG, 1:513])
        nc.gpsimd.dma_start(out_img(r0 + 2), osv[:, GIMG:NSEG, 1:513])


EP = 96   # edge partitions: p = chunk*24 + (edge*12 + img), 4 col-chunks
EW = 130  # edge grid width; position j of chunk c holds col 128c-1+j


def _edge_loads(nc, pio, xi):
    """Loads for image rows 0 and 511, reshaped to [96, 130]: each of the
    24 (edge,img) rows is split into 4 column-chunks of 128 with a 1-col
    halo on each side; the image-boundary halo positions are zeroed, so
    the pad columns fold into the grid and the edge compute needs NO
    boundary special-casing. Issued up front (tiny) so the end-of-kernel
    edge compute never waits on DMA."""
    R0 = pio.tile([EP, EW], BF16, tag="R0", name="R0")
    R1 = pio.tile([EP, EW], BF16, tag="R1", name="R1")
    for T in (R0, R1):
        # zero both halo columns on all partitions (the verifier rejects
        # partition ranges not starting at 0); loads then overwrite the
        # non-pad ones, leaving zeros only at chunk 0 col -1 / chunk 3
        # col 512
        nc.gpsimd.memset(T[:][0:EP, 0:1], 0.0)
        nc.gpsimd.memset(T[:][0:EP, 129:130], 0.0)
    for T, rows in ((R0, (0, H - 1)), (R1, (1, H - 2))):
        for k, r in enumerate(rows):  # k=0: slots 0..11, k=1: slots 12..23
            q = nc.sync if k == 0 else nc.scalar
            for c in range(4):
                p0 = 24 * c + 12 * k
                a, b = (1, 130) if c == 0 else (0, 129) if c == 3 else (0, 130)
                cols = slice(128 * c - 1 + a, 128 * c - 1 + b)
                q.dma_start(T[:][p0 : p0 + 12, a:b], xi[:, r, cols])
    return R0, R1


def _edge_compute(nc, pio, pm, oi, R0, R1):
    """Rows 0 and 511 (windows contain the zero pad row). Runs last, in
    the shadow of the final block's output stores."""
    NI = EW - 2  # 128

    def t2(tag):
        return pm.tile([EP, EW], BF16, tag=f"e{tag}", name=f"e{tag}")

    rmn, rmx = t2("rmn"), t2("rmx")
    tt = nc.vector.tensor_tensor
    tt(rmn[:], R0[:], R1[:], op=MIN)
    tt(rmx[:], R0[:], R1[:], op=MAX)

    # vertical sort3 with the zero pad row: min/max vs 0, med=max(mn,min(mx,0))
    MN0, MD0, MX0 = t2("MN"), t2("MD"), t2("MX")
    nc.vector.tensor_scalar_min(MN0[:], rmn[:], 0.0)
    nc.vector.tensor_scalar_max(MX0[:], rmx[:], 0.0)
    nc.vector.scalar_tensor_tensor(MD0[:], rmx[:], 0.0, rmn[:], op0=MIN, op1=MAX)

    s1 = {}
    for name, F in (("MN", MN0), ("MD", MD0), ("MX", MX0)):
        T = t2(f"s1{name}")
        nc.scalar.copy(T[:][0:EP, 0 : EW - 1], F[:][0:EP, 1:EW])
        s1[name] = T

    fl = lambda T, a, b: T[:][0:EP, a:b]
    Pmn, Pmx, Qmn, Qmx = t2("Pmn"), t2("Pmx"), t2("Qmn"), t2("Qmx")
    tmd, Rmn, Rmd, Rmx = t2("tmd"), t2("Rmn"), t2("Rmd"), t2("Rmx")
    MN1, MX1, TF = t2("MN1"), t2("MX1"), t2("TF")
    OUT0 = pio.tile([EP, EW], BF16, tag="OUT0", name="OUT0")

    tt(fl(Pmn, 0, NI), fl(MN0, 0, NI), fl(s1["MN"], 0, NI), op=MAX)
    tt(fl(Qmn, 0, NI), fl(MD0, 0, NI), fl(s1["MD"], 0, NI), op=MIN)
    tt(fl(Qmx, 0, NI), fl(MD0, 0, NI), fl(s1["MD"], 0, NI), op=MAX)
    tt(fl(Pmx, 0, NI), fl(MX0, 0, NI), fl(s1["MX"], 0, NI), op=MIN)
    tt(fl(Rmn, 0, NI), fl(Pmn, 0, NI), fl(MN0, 2, EW), op=MAX)
    tt(fl(tmd, 0, NI), fl(Qmx, 0, NI), fl(MD0, 2, EW), op=MIN)
    tt(fl(Rmd, 0, NI), fl(Qmn, 0, NI), fl(tmd, 0, NI), op=MAX)
    tt(fl(Rmx, 0, NI), fl(Pmx, 0, NI), fl(MX0, 2, EW), op=MIN)
    tt(fl(MN1, 0, NI), fl(Rmn, 0, NI), fl(Rmd, 0, NI), op=MIN)
    tt(fl(MX1, 0, NI), fl(Rmn, 0, NI), fl(Rmd, 0, NI), op=MAX)
    tt(fl(TF, 0, NI), fl(MX1, 0, NI), fl(Rmx, 0, NI), op=MIN)
    tt(fl(OUT0, 2, EW), fl(MN1, 0, NI), fl(TF, 0, NI), op=MAX)

    # SWDGE is empty by now (its last work was the mid-kernel stores), so
    # these tiny final stores' completion overlaps the HWDGE stores' ack
    for k, r in enumerate((0, H - 1)):
        for c in range(4):
            p0 = 24 * c + 12 * k
            nc.gpsimd.dma_start(
                oi[:, r, 128 * c : 128 * c + 128],
                OUT0[:][p0 : p0 + 12, 2:EW],
            )


def build_program():
    nc = bacc.Bacc(
        "TRN2", target_bir_lowering=False, debug=False, num_devices=N_CORES
    )
    x_d = nc.dram_tensor("x", [B_PER, C, H, W], BF16, kind="ExternalInput").ap()
    o_d = nc.dram_tensor("out", [B_PER, C, H, W], BF16, kind="ExternalOutput").ap()
    xh = x_d.rearrange("b c h w -> h (b c) w")  # [512, 12, 512]
    oh = o_d.rearrange("b c h w -> h (b c) w")
    xi = x_d.rearrange("b c h w -> (b c) h w")  # [12, 512, 512]
    oi = o_d.rearrange("b c h w -> (b c) h w")

    with tile.TileContext(nc) as tc:
        with (
            tc.tile_pool(name="io", bufs=1) as pio,
            tc.tile_pool(name="mid", bufs=1) as pm,
        ):
            nb = 2 * (NIMG // GIMG)
            _block(nc, pio, pm, xh, oh, 0, 0, first=True)
            # edge loads are tiny; edge COMPUTE runs last, in the shadow of
            # the final block's output stores
            R0, R1 = _edge_loads(nc, pio, xi)
            for i in range(1, nb):
                _block(nc, pio, pm, xh, oh, i // 2, i % 2, last=(i == nb - 1))
            _edge_compute(nc, pio, pm, oi, R0, R1)
    nc.compile()
    return nc


def _get_program():
    global _PROGRAM
    if _PROGRAM is None:
        _PROGRAM = build_program()
    return _PROGRAM


def make_in_maps(x: np.ndarray):
    xb = np.ascontiguousarray(x).astype(ml_dtypes.bfloat16)
    return [{"x": xb[k * B_PER : (k + 1) * B_PER]} for k in range(N_CORES)]


def kernel(**inputs) -> np.ndarray:
    x = np.asarray(inputs["x"], dtype=np.float32)
    assert x.shape == (B, C, H, W), x.shape
    nc = _get_program()
    res = bass_utils.run_bass_kernel_spmd(
        nc, make_in_maps(x), core_ids=list(range(N_CORES))
    )
    out = np.concatenate(
        [np.asarray(res.results[k]["out"]) for k in range(N_CORES)], axis=0
    )
    return out.astype(np.float32)


# revision 28
# speedup vs baseline: 1.0015x; 1.0015x over previous
"""3x3 median filter (zero-padded) on TRN2, 8 NeuronCores, bf16 datapath.

Input  x: (32, 3, 512, 512) float32
Output  : (32, 3, 512, 512) float32.

Accuracy: the median network only ever SELECTS one of its 9 inputs (min/max
ops create no new values), so the device-side bf16 result equals the bf16
rounding of the element that is the median of the rounded window. Order
statistics are 1-Lipschitz under sup-norm perturbation, so end-to-end error
is <= 2^-8 relative -- far inside the 2e-2 gate. Measured 3.4e-3.

Strategy
--------
Pure data parallel: batch dim sharded 4-per-core across 8 cores; per core
12 images (4 batch x 3 chan) in 2 groups of 6 images x 2 vertical halves.

bf16 doubles DVE tensor_tensor throughput (2x_1P perf mode) but ONLY for
unit-stride 4-byte-aligned access patterns, so the horizontal stage is
restructured from the fp32 baseline's stride-2 parity tricks into dense
shifted-field form. Per field F in {MN,MD,MX}: s1F[j]=F[j+1] is the ONLY
odd-element shift (a ScalarE copy -- ACT is otherwise idle), then
   P[j] = op(F[j], s1F[j])          # aligned TT, 2x
   R[j] = op(P[j], F[j+2])          # +2 elems = 4B-aligned shift, 2x
R[j] = sliding-3 result centered at col j+1; the final med3(Rmn,Rmd,Rmx)
writes into an OUT grid whose per-image segment holds col c at position
c+1, so the wide write starts at even offset 2 and the DMA store (which
doesn't care about alignment) un-shifts.

Both row parities' fields live in ONE fused [128, 12*514] tile (odd-parity
images = segments 0..5, even = 6..11), so stage 2 is 12 double-width ops
per block instead of 24 -- per-op overhead (58-cycle issue + ~90ns DRAIN)
is the only thing that changes, the streamed cycles are identical. Output
cols 0 and 511 (windows containing the zero pad column) are 7 tiny ops on
gathered grid positions {0,510} across all 12 segments.

Grid: per-image segment width 514 (even -> every segment start keeps 4B
parity). Segment positions 512..513 are scratch: stage-1 ops run flat over
the whole grid and compute garbage there; no stored output reads a garbage
lane (out cols 0/511 come from the boundary path).

SBUF fits via aggressive aliasing with DVE-program-order-safe lifetimes:
stage-1's qmn/qmx/t_o/t_e live in halves of stage-2's Pmx/Qmx slots;
stage-2's Rmx/tmd/Rmd overwrite the dead MN/MX/MD field buffers; MN1/MX1/TF
overwrite Pmn/Qmn/Qmx. Stage-1 emits fields in MN,MX,MD order and ACT
copies s1MN,s1MX,s1MD in that order so every copy lands before the DVE
needs it, with no stall.

Image rows 0 and 511 (windows contain the zero pad row): one small
24-partition pass issued LAST so it fills the DVE-idle tail while the
final block's output stores (sent to the idle HWDGE queues) drain. Its
tiny loads are issued up front. Block 0's loads+stage-1 ramp up in 1+2+3
image chunks sized to the ~250GB/s strided-row-gather DMA rate, so the
DVE starts ~2us after the first 0.25MB lands instead of waiting for 3MB.

Engine budget per core: DVE 17 TT/elem at 2x ~= 236us busy (the floor for
this decomposition), ACT ~75us, DMA ~19MB. Loads on the SP+ACT HWDGE
queues, mid-kernel stores on the GpSimd SWDGE queue.
"""
import sys

if "/opt/trn_rl_repo" not in sys.path:
    sys.path.insert(0, "/opt/trn_rl_repo")

import numpy as np
import ml_dtypes
import concourse.bacc as bacc
import concourse.mybir as mybir
import concourse.tile as tile
from concourse import bass_utils

B, C, H, W = 32, 3, 512, 512
N_CORES = 8
B_PER = B // N_CORES          # 4 batches per core
NIMG = B_PER * C              # 12 images per core
GIMG = 6                      # images per tile group
PW = W + 2                    # per-image grid width (514, even)
FP = GIMG * PW                # half (one parity) grid width (3084)
FP2 = 2 * FP                  # fused two-parity grid width (6168)
NSEG = 2 * GIMG               # segments in a fused tile (12)
HH = H // 2                   # 256 rows per vertical half
P = 128                       # partitions = row pairs per half
NE = 2 * NIMG                 # partitions in the edge-rows pass (24)

BF16 = mybir.dt.bfloat16
MIN = mybir.AluOpType.min
MAX = mybir.AluOpType.max

_PROGRAM = None


def _seg(T, npart, nseg):
    """[npart, nseg, 514] per-image-segment view."""
    return T[:].rearrange("p (i w) -> p i w", w=PW)[0:npart, 0:nseg]


def _stage2_fused(nc, pm, MN, MD, MX, OUT, stores=None):
    """Both parities of stage 2 in double-width ops on the fused grids.
    ScalarE does the three s1 shifts (issued in MN,MX,MD order to match
    the DVE's consumption order). Aliases overwrite only dead buffers.
    stores: optional per-parity store callbacks; when given, the final op
    and boundary write are split per parity and each parity's store is
    issued the moment its OUT half is complete (used by the last block to
    shrink the end-of-kernel store drain)."""
    NI = FP2 - 2
    fl = lambda T, a, b: T[:][:, a:b]

    s1 = {}
    for nm, F in (("MN", MN), ("MX", MX), ("MD", MD)):
        T = pm.tile([P, FP2], BF16, tag=f"s1{nm}", name=f"s1{nm}")
        nc.scalar.copy(T[:][:, 0 : FP2 - 1], F[:][:, 1:FP2])
        s1[nm] = T

    def t2(tag):
        return pm.tile([P, FP2], BF16, tag=tag, name=tag)

    def alias(tag, name):
        return pm.tile([P, FP2], BF16, tag=tag, name=name)

    Pmn, Pmx, Qmn, Qmx, Rmn = t2("Pmn"), t2("Pmx"), t2("Qmn"), t2("Qmx"), t2("Rmn")
    Rmx = alias("fMN", "Rmx")   # MN field dead after Rmn
    tmd = alias("fMX", "tmd")   # MX field dead after Rmx
    Rmd = alias("fMD", "Rmd")   # MD field dead after tmd
    MN1 = alias("Pmn", "MN1")
    MX1 = alias("Qmn", "MX1")
    TF = alias("Qmx", "TF")

    tt = nc.vector.tensor_tensor
    gv = lambda T: _seg(T, P, NSEG)[:, :, 0:511:510]
    bt = lambda tag: pm.tile([P, 2 * NSEG], BF16, tag=f"{tag}b", name=f"{tag}b")
    bv = lambda T: T[:].rearrange("p (i c) -> p i c", c=2)[0:P, 0:NSEG]
    BA, BB, BC = bt("BA"), bt("BB"), bt("BC")
    B1, B2, B3 = bt("B1"), bt("B2"), bt("B3")

    tt(fl(Pmn, 0, NI), fl(MN, 0, NI), fl(s1["MN"], 0, NI), op=MAX)
    tt(fl(Pmx, 0, NI), fl(MX, 0, NI), fl(s1["MX"], 0, NI), op=MIN)
    nc.vector.tensor_scalar_max(bv(BA), gv(Pmn), 0.0)
    nc.vector.tensor_scalar_min(bv(BC), gv(Pmx), 0.0)
    tt(fl(Rmn, 0, NI), fl(Pmn, 0, NI), fl(MN, 2, FP2), op=MAX)
    tt(fl(Rmx, 0, NI), fl(Pmx, 0, NI), fl(MX, 2, FP2), op=MIN)
    tt(fl(Qmn, 0, NI), fl(MD, 0, NI), fl(s1["MD"], 0, NI), op=MIN)
    tt(fl(Qmx, 0, NI), fl(MD, 0, NI), fl(s1["MD"], 0, NI), op=MAX)
    nc.vector.scalar_tensor_tensor(bv(BB), gv(Qmx), 0.0, gv(Qmn), op0=MIN, op1=MAX)
    tt(fl(tmd, 0, NI), fl(Qmx, 0, NI), fl(MD, 2, FP2), op=MIN)
    tt(fl(Rmd, 0, NI), fl(Qmn, 0, NI), fl(tmd, 0, NI), op=MAX)
    # med3(Rmn, Rmd, Rmx); last wide op writes output cols 1..510 at grid
    # positions 2..511 (even start -> stays in 2x mode)
    tt(fl(MN1, 0, NI), fl(Rmn, 0, NI), fl(Rmd, 0, NI), op=MIN)
    tt(fl(MX1, 0, NI), fl(Rmn, 0, NI), fl(Rmd, 0, NI), op=MAX)
    tt(fl(TF, 0, NI), fl(MX1, 0, NI), fl(Rmx, 0, NI), op=MIN)
    # boundary med3 staging (tiny)
    tt(bv(B1), bv(BA), bv(BB), op=MIN)
    tt(bv(B2), bv(BA), bv(BB), op=MAX)
    tt(bv(B3), bv(B2), bv(BC), op=MIN)

    def final(s0, s1_):
        ov = _seg(OUT, P, NSEG)[:, s0:s1_, 2:512]
        tt(ov, _seg(MN1, P, NSEG)[:, s0:s1_, 0:510],
           _seg(TF, P, NSEG)[:, s0:s1_, 0:510], op=MAX)
        tt(_seg(OUT, P, NSEG)[:, s0:s1_, 1:513:511],
           bv(B1)[:, s0:s1_], bv(B3)[:, s0:s1_], op=MAX)

    if stores is None:
        final(0, NSEG)
    else:
        for h, cb in enumerate(stores):
            final(h * GIMG, (h + 1) * GIMG)
            cb()


def _block(nc, pio, pm, xh, oh, g, half, first=False, last=False):
    """One vertical half of one image group: odd output rows r0+1..r0+255,
    even rows r0+2..r0+256 (halves overlap by two rows so every DMA is a
    full 128-partition transfer). Rows 0 and 511 via the edge pass.
    first=True: loads and stage 1 ramp in 1+2+3 image chunks (cold start).
    last=True: stores go to the by-then-idle HWDGE queues so the final
    drain overlaps the edge pass."""
    r0 = 0 if half == 0 else H - HH - 2
    i0 = GIMG * g

    E = pio.tile([P, FP], BF16, tag="E", name="E")
    O = pio.tile([P, FP], BF16, tag="O", name="O")
    E_sh = pio.tile([P, FP], BF16, tag="E_sh", name="E_sh")
    O_sh2 = pio.tile([P, FP], BF16, tag="O_sh2", name="O_sh2")

    # scratch cols 512..513 of each segment are read by the flat stage-1
    # ops but never loaded: define them so no lane is uninitialized
    for T in (E, O, E_sh, O_sh2):
        nc.gpsimd.memset(_seg(T, P, GIMG)[:, :, W:PW], 0.0)

    def loads(ia, ib):
        lv = lambda T: _seg(T, P, GIMG)[:, ia:ib, 0:W]
        im = lambda r_lo: xh[
            r_lo : min(r_lo + 2 * P, H) : 2, i0 + ia : i0 + ib, :
        ]
        # queue order matters (HWDGE queues are FIFOs): the (O, E_sh) pair
        # feeds the first op of the block, so those loads go first
        nc.sync.dma_start(lv(E_sh), im(r0 + 2))     # rows r0+2p+2
        nc.scalar.dma_start(lv(O), im(r0 + 1))      # rows r0+2p+1
        nc.sync.dma_start(lv(E), im(r0))            # rows r0+2p
        nc.scalar.dma_start(lv(O_sh2), im(r0 + 3))  # rows r0+2p+3

    # stage-1 temps live in halves of stage-2 slots that are written later
    qpair = pm.tile([P, FP2], BF16, tag="Pmx", name="qpair")
    tpair = pm.tile([P, FP2], BF16, tag="Qmx", name="tpair")
    MN2 = pm.tile([P, FP2], BF16, tag="fMN", name="fMN")
    MD2 = pm.tile([P, FP2], BF16, tag="fMD", name="fMD")
    MX2 = pm.tile([P, FP2], BF16, tag="fMX", name="fMX")

    tt = nc.vector.tensor_tensor

    def stage1(ia, ib):
        a, b = ia * PW, ib * PW
        qv = lambda h: qpair[:][:, h * FP + a : h * FP + b]
        tv = lambda h: tpair[:][:, h * FP + a : h * FP + b]
        f = lambda T, h: T[:][:, h * FP + a : h * FP + b]
        sv = lambda T: T[:][:, a:b]
        qmn, qmx = qv(0), qv(1)
        # shared pair = (O, E_sh) = rows (2p+1, 2p+2)
        tt(qmn, sv(O), sv(E_sh), op=MIN)
        tt(qmx, sv(O), sv(E_sh), op=MAX)
        # field completion order MN, MX, MD matches the ACT copy order in
        # _stage2_fused so no s1 copy ever stalls the DVE.
        # odd output rows r0+2p+1: pair + E; even rows: pair + O_sh2
        tt(f(MN2, 0), qmn, sv(E), op=MIN)
        tt(f(MN2, 1), qmn, sv(O_sh2), op=MIN)
        tt(f(MX2, 0), qmx, sv(E), op=MAX)
        tt(f(MX2, 1), qmx, sv(O_sh2), op=MAX)
        tt(tv(0), qmx, sv(E), op=MIN)
        tt(f(MD2, 0), qmn, tv(0), op=MAX)
        tt(tv(1), qmx, sv(O_sh2), op=MIN)
        tt(f(MD2, 1), qmn, tv(1), op=MAX)

    if first:
        for ia, ib in ((0, 1), (1, 3), (3, 6)):
            loads(ia, ib)
            stage1(ia, ib)
    else:
        loads(0, GIMG)
        stage1(0, GIMG)

    OUT = pio.tile([P, FP2], BF16, tag="OUT", name="OUT")
    out_img = lambda r_lo: oh[r_lo : min(r_lo + 2 * P, H) : 2, i0 : i0 + GIMG, :]
    hi = GIMG // 2
    osv = _seg(OUT, P, NSEG)
    if last:
        # the end-of-kernel critical path is (last stores + HBM write ack):
        # parity o stores on SWDGE the moment its OUT half is done, and
        # parity e drains as two parallel half-stores on the idle HWDGE
        # queues right after its final op
        def store_o():
            nc.gpsimd.dma_start(out_img(r0 + 1), osv[:, 0:GIMG, 1:513])

        def store_e():
            oe = oh[r0 + 2 : min(r0 + 2 + 2 * P, H) : 2, i0 : i0 + GIMG, :]
            nc.sync.dma_start(oe[:, 0:hi], osv[:, GIMG : GIMG + hi, 1:513])
            nc.scalar.dma_start(oe[:, hi:GIMG],
                                osv[:, GIMG + hi : NSEG, 1:513])

        _stage2_fused(nc, pm, MN2, MD2, MX2, OUT, stores=(store_o, store_e))
    else:
        _stage2_fused(nc, pm, MN2, MD2, MX2, OUT)
        # stores on the SWDGE queue so they never block later blocks' loads
        nc.gpsimd.dma_start(out_img(r0 + 1), osv[:, 0:GIMG, 1:513])
        nc.gpsimd.dma_start(out_img(r0 + 2), osv[:, GIMG:NSEG, 1:513])


EP = 96   # edge partitions: p = chunk*24 + (edge*12 + img), 4 col-chunks
EW = 130  # edge grid width; position j of chunk c holds col 128c-1+j


def _edge_loads(nc, pio, xi):
    """Loads for image rows 0 and 511, reshaped to [96, 130]: each of the
    24 (edge,img) rows is split into 4 column-chunks of 128 with a 1-col
    halo on each side; the image-boundary halo positions are zeroed, so
    the pad columns fold into the grid and the edge compute needs NO
    boundary special-casing. Issued up front (tiny) so the end-of-kernel
    edge compute never waits on DMA."""
    R0 = pio.tile([EP, EW], BF16, tag="R0", name="R0")
    R1 = pio.tile([EP, EW], BF16, tag="R1", name="R1")
    for T in (R0, R1):
        # zero both halo columns on all partitions (the verifier rejects
        # partition ranges not starting at 0); loads then overwrite the
        # non-pad ones, leaving zeros only at chunk 0 col -1 / chunk 3
        # col 512
        nc.gpsimd.memset(T[:][0:EP, 0:1], 0.0)
        nc.gpsimd.memset(T[:][0:EP, 129:130], 0.0)
    for T, rows in ((R0, (0, H - 1)), (R1, (1, H - 2))):
        for k, r in enumerate(rows):  # k=0: slots 0..11, k=1: slots 12..23
            q = nc.sync if k == 0 else nc.scalar
            for c in range(4):
                p0 = 24 * c + 12 * k
                a, b = (1, 130) if c == 0 else (0, 129) if c == 3 else (0, 130)
                cols = slice(128 * c - 1 + a, 128 * c - 1 + b)
                q.dma_start(T[:][p0 : p0 + 12, a:b], xi[:, r, cols])
    return R0, R1


def _edge_compute(nc, pio, pm, oi, R0, R1):
    """Rows 0 and 511 (windows contain the zero pad row). Runs last, in
    the shadow of the final block's output stores."""
    NI = EW - 2  # 128

    def t2(tag):
        return pm.tile([EP, EW], BF16, tag=f"e{tag}", name=f"e{tag}")

    rmn, rmx = t2("rmn"), t2("rmx")
    tt = nc.vector.tensor_tensor
    tt(rmn[:], R0[:], R1[:], op=MIN)
    tt(rmx[:], R0[:], R1[:], op=MAX)

    # vertical sort3 with the zero pad row: min/max vs 0, med=max(mn,min(mx,0))
    MN0, MD0, MX0 = t2("MN"), t2("MD"), t2("MX")
    nc.vector.tensor_scalar_min(MN0[:], rmn[:], 0.0)
    nc.vector.tensor_scalar_max(MX0[:], rmx[:], 0.0)
    nc.vector.scalar_tensor_tensor(MD0[:], rmx[:], 0.0, rmn[:], op0=MIN, op1=MAX)

    # shifts on the DVE itself (2x_2P copies, ~0.2us each): the edge pass
    # must not touch the scalar engine, whose queue holds the LAST block's
    # loads right behind these instructions
    s1 = {}
    for name, F in (("MN", MN0), ("MD", MD0), ("MX", MX0)):
        T = t2(f"s1{name}")
        nc.vector.tensor_copy(T[:][0:EP, 0 : EW - 1], F[:][0:EP, 1:EW])
        s1[name] = T

    fl = lambda T, a, b: T[:][0:EP, a:b]
    Pmn, Pmx, Qmn, Qmx = t2("Pmn"), t2("Pmx"), t2("Qmn"), t2("Qmx")
    tmd, Rmn, Rmd, Rmx = t2("tmd"), t2("Rmn"), t2("Rmd"), t2("Rmx")
    MN1, MX1, TF = t2("MN1"), t2("MX1"), t2("TF")
    OUT0 = pio.tile([EP, EW], BF16, tag="OUT0", name="OUT0")

    tt(fl(Pmn, 0, NI), fl(MN0, 0, NI), fl(s1["MN"], 0, NI), op=MAX)
    tt(fl(Qmn, 0, NI), fl(MD0, 0, NI), fl(s1["MD"], 0, NI), op=MIN)
    tt(fl(Qmx, 0, NI), fl(MD0, 0, NI), fl(s1["MD"], 0, NI), op=MAX)
    tt(fl(Pmx, 0, NI), fl(MX0, 0, NI), fl(s1["MX"], 0, NI), op=MIN)
    tt(fl(Rmn, 0, NI), fl(Pmn, 0, NI), fl(MN0, 2, EW), op=MAX)
    tt(fl(tmd, 0, NI), fl(Qmx, 0, NI), fl(MD0, 2, EW), op=MIN)
    tt(fl(Rmd, 0, NI), fl(Qmn, 0, NI), fl(tmd, 0, NI), op=MAX)
    tt(fl(Rmx, 0, NI), fl(Pmx, 0, NI), fl(MX0, 2, EW), op=MIN)
    tt(fl(MN1, 0, NI), fl(Rmn, 0, NI), fl(Rmd, 0, NI), op=MIN)
    tt(fl(MX1, 0, NI), fl(Rmn, 0, NI), fl(Rmd, 0, NI), op=MAX)
    tt(fl(TF, 0, NI), fl(MX1, 0, NI), fl(Rmx, 0, NI), op=MIN)
    tt(fl(OUT0, 2, EW), fl(MN1, 0, NI), fl(TF, 0, NI), op=MAX)

    # SWDGE is empty by now (its last work was the mid-kernel stores), so
    # these tiny final stores' completion overlaps the HWDGE stores' ack
    for k, r in enumerate((0, H - 1)):
        for c in range(4):
            p0 = 24 * c + 12 * k
            nc.gpsimd.dma_start(
                oi[:, r, 128 * c : 128 * c + 128],
                OUT0[:][p0 : p0 + 12, 2:EW],
            )


def build_program():
    nc = bacc.Bacc(
        "TRN2", target_bir_lowering=False, debug=False, num_devices=N_CORES
    )
    x_d = nc.dram_tensor("x", [B_PER, C, H, W], BF16, kind="ExternalInput").ap()
    o_d = nc.dram_tensor("out", [B_PER, C, H, W], BF16, kind="ExternalOutput").ap()
    xh = x_d.rearrange("b c h w -> h (b c) w")  # [512, 12, 512]
    oh = o_d.rearrange("b c h w -> h (b c) w")
    xi = x_d.rearrange("b c h w -> (b c) h w")  # [12, 512, 512]
    oi = o_d.rearrange("b c h w -> (b c) h w")

    with tile.TileContext(nc) as tc:
        with (
            tc.tile_pool(name="io", bufs=1) as pio,
            tc.tile_pool(name="mid", bufs=1) as pm,
        ):
            nb = 2 * (NIMG // GIMG)
            _block(nc, pio, pm, xh, oh, 0, 0, first=True)
            R0, R1 = _edge_loads(nc, pio, xi)
            for i in range(1, nb - 1):
                _block(nc, pio, pm, xh, oh, i // 2, i % 2)
            # edge pass second-to-last: its stores drain under the final
            # block's compute, leaving only that block's (split) stores +
            # HBM write-ack on the end-of-kernel critical path
            _edge_compute(nc, pio, pm, oi, R0, R1)
            _block(nc, pio, pm, xh, oh, (nb - 1) // 2, (nb - 1) % 2,
                   last=True)
    nc.compile()
    return nc


def _get_program():
    global _PROGRAM
    if _PROGRAM is None:
        _PROGRAM = build_program()
    return _PROGRAM


def make_in_maps(x: np.ndarray):
    xb = np.ascontiguousarray(x).astype(ml_dtypes.bfloat16)
    return [{"x": xb[k * B_PER : (k + 1) * B_PER]} for k in range(N_CORES)]


def kernel(**inputs) -> np.ndarray:
    x = np.asarray(inputs["x"], dtype=np.float32)
    assert x.shape == (B, C, H, W), x.shape
    nc = _get_program()
    res = bass_utils.run_bass_kernel_spmd(
        nc, make_in_maps(x), core_ids=list(range(N_CORES))
    )
    out = np.concatenate(
        [np.asarray(res.results[k]["out"]) for k in range(N_CORES)], axis=0
    )
    return out.astype(np.float32)


# revision 31
# speedup vs baseline: 1.0033x; 1.0018x over previous
"""3x3 median filter (zero-padded) on TRN2, 8 NeuronCores, bf16 datapath.

Input  x: (32, 3, 512, 512) float32
Output  : (32, 3, 512, 512) float32.

Accuracy: the median network only ever SELECTS one of its 9 inputs (min/max
ops create no new values), so the device-side bf16 result equals the bf16
rounding of the element that is the median of the rounded window. Order
statistics are 1-Lipschitz under sup-norm perturbation, so end-to-end error
is <= 2^-8 relative -- far inside the 2e-2 gate. Measured 3.4e-3.

Strategy
--------
Pure data parallel: batch dim sharded 4-per-core across 8 cores; per core
12 images (4 batch x 3 chan) in 2 groups of 6 images x 2 vertical halves.

bf16 doubles DVE tensor_tensor throughput (2x_1P perf mode) but ONLY for
unit-stride 4-byte-aligned access patterns, so the horizontal stage is
restructured from the fp32 baseline's stride-2 parity tricks into dense
shifted-field form. Per field F in {MN,MD,MX}: s1F[j]=F[j+1] is the ONLY
odd-element shift (a ScalarE copy -- ACT is otherwise idle), then
   P[j] = op(F[j], s1F[j])          # aligned TT, 2x
   R[j] = op(P[j], F[j+2])          # +2 elems = 4B-aligned shift, 2x
R[j] = sliding-3 result centered at col j+1; the final med3(Rmn,Rmd,Rmx)
writes into an OUT grid whose per-image segment holds col c at position
c+1, so the wide write starts at even offset 2 and the DMA store (which
doesn't care about alignment) un-shifts.

Both row parities' fields live in ONE fused [128, 12*514] tile (odd-parity
images = segments 0..5, even = 6..11), so stage 2 is 12 double-width ops
per block instead of 24 -- per-op overhead (58-cycle issue + ~90ns DRAIN)
is the only thing that changes, the streamed cycles are identical. Output
cols 0 and 511 (windows containing the zero pad column) are 7 tiny ops on
gathered grid positions {0,510} across all 12 segments.

Grid: per-image segment width 514 (even -> every segment start keeps 4B
parity). Segment positions 512..513 are scratch: stage-1 ops run flat over
the whole grid and compute garbage there; no stored output reads a garbage
lane (out cols 0/511 come from the boundary path).

SBUF fits via aggressive aliasing with DVE-program-order-safe lifetimes:
stage-1's qmn/qmx/t_o/t_e live in halves of stage-2's Pmx/Qmx slots;
stage-2's Rmx/tmd/Rmd overwrite the dead MN/MX/MD field buffers; MN1/MX1/TF
overwrite Pmn/Qmn/Qmx. Stage-1 emits fields in MN,MX,MD order and ACT
copies s1MN,s1MX,s1MD in that order so every copy lands before the DVE
needs it, with no stall.

Image rows 0 and 511 (windows contain the zero pad row): one small
24-partition pass issued LAST so it fills the DVE-idle tail while the
final block's output stores (sent to the idle HWDGE queues) drain. Its
tiny loads are issued up front. Block 0's loads+stage-1 ramp up in 1+2+3
image chunks sized to the ~250GB/s strided-row-gather DMA rate, so the
DVE starts ~2us after the first 0.25MB lands instead of waiting for 3MB.

Engine budget per core: DVE 17 TT/elem at 2x ~= 236us busy (the floor for
this decomposition), ACT ~75us, DMA ~19MB. Loads on the SP+ACT HWDGE
queues, mid-kernel stores on the GpSimd SWDGE queue.
"""
import sys

if "/opt/trn_rl_repo" not in sys.path:
    sys.path.insert(0, "/opt/trn_rl_repo")

import numpy as np
import ml_dtypes
import concourse.bacc as bacc
import concourse.mybir as mybir
import concourse.tile as tile
from concourse import bass_utils

B, C, H, W = 32, 3, 512, 512
N_CORES = 8
B_PER = B // N_CORES          # 4 batches per core
NIMG = B_PER * C              # 12 images per core
GIMG = 6                      # images per tile group
PW = W + 2                    # per-image grid width (514, even)
FP = GIMG * PW                # half (one parity) grid width (3084)
FP2 = 2 * FP                  # fused two-parity grid width (6168)
NSEG = 2 * GIMG               # segments in a fused tile (12)
HH = H // 2                   # 256 rows per vertical half
P = 128                       # partitions = row pairs per half
NE = 2 * NIMG                 # partitions in the edge-rows pass (24)

BF16 = mybir.dt.bfloat16
MIN = mybir.AluOpType.min
MAX = mybir.AluOpType.max

_PROGRAM = None


def _seg(T, npart, nseg):
    """[npart, nseg, 514] per-image-segment view."""
    return T[:].rearrange("p (i w) -> p i w", w=PW)[0:npart, 0:nseg]


def _stage2_fused(nc, pm, MN, MD, MX, OUT, stores=None):
    """Both parities of stage 2 in double-width ops on the fused grids.
    ScalarE does the three s1 shifts (issued in MN,MX,MD order to match
    the DVE's consumption order). Aliases overwrite only dead buffers.
    stores: optional per-parity store callbacks; when given, the final op
    and boundary write are split per parity and each parity's store is
    issued the moment its OUT half is complete (used by the last block to
    shrink the end-of-kernel store drain)."""
    NI = FP2 - 2
    fl = lambda T, a, b: T[:][:, a:b]

    s1 = {}
    for nm, F in (("MN", MN), ("MX", MX), ("MD", MD)):
        T = pm.tile([P, FP2], BF16, tag=f"s1{nm}", name=f"s1{nm}")
        nc.scalar.copy(T[:][:, 0 : FP2 - 1], F[:][:, 1:FP2])
        s1[nm] = T

    def t2(tag):
        return pm.tile([P, FP2], BF16, tag=tag, name=tag)

    def alias(tag, name):
        return pm.tile([P, FP2], BF16, tag=tag, name=name)

    Pmn, Pmx, Qmn, Qmx, Rmn = t2("Pmn"), t2("Pmx"), t2("Qmn"), t2("Qmx"), t2("Rmn")
    Rmx = alias("fMN", "Rmx")   # MN field dead after Rmn
    tmd = alias("fMX", "tmd")   # MX field dead after Rmx
    Rmd = alias("fMD", "Rmd")   # MD field dead after tmd
    MN1 = alias("Pmn", "MN1")
    MX1 = alias("Qmn", "MX1")
    TF = alias("Qmx", "TF")

    tt = nc.vector.tensor_tensor
    gv = lambda T: _seg(T, P, NSEG)[:, :, 0:511:510]
    bt = lambda tag: pm.tile([P, 2 * NSEG], BF16, tag=f"{tag}b", name=f"{tag}b")
    bv = lambda T: T[:].rearrange("p (i c) -> p i c", c=2)[0:P, 0:NSEG]
    BA, BB, BC = bt("BA"), bt("BB"), bt("BC")
    B1, B2, B3 = bt("B1"), bt("B2"), bt("B3")

    tt(fl(Pmn, 0, NI), fl(MN, 0, NI), fl(s1["MN"], 0, NI), op=MAX)
    tt(fl(Pmx, 0, NI), fl(MX, 0, NI), fl(s1["MX"], 0, NI), op=MIN)
    nc.vector.tensor_scalar_max(bv(BA), gv(Pmn), 0.0)
    nc.vector.tensor_scalar_min(bv(BC), gv(Pmx), 0.0)
    tt(fl(Rmn, 0, NI), fl(Pmn, 0, NI), fl(MN, 2, FP2), op=MAX)
    tt(fl(Rmx, 0, NI), fl(Pmx, 0, NI), fl(MX, 2, FP2), op=MIN)
    tt(fl(Qmn, 0, NI), fl(MD, 0, NI), fl(s1["MD"], 0, NI), op=MIN)
    tt(fl(Qmx, 0, NI), fl(MD, 0, NI), fl(s1["MD"], 0, NI), op=MAX)
    nc.vector.scalar_tensor_tensor(bv(BB), gv(Qmx), 0.0, gv(Qmn), op0=MIN, op1=MAX)
    tt(fl(tmd, 0, NI), fl(Qmx, 0, NI), fl(MD, 2, FP2), op=MIN)
    tt(fl(Rmd, 0, NI), fl(Qmn, 0, NI), fl(tmd, 0, NI), op=MAX)
    # med3(Rmn, Rmd, Rmx); last wide op writes output cols 1..510 at grid
    # positions 2..511 (even start -> stays in 2x mode)
    tt(fl(MN1, 0, NI), fl(Rmn, 0, NI), fl(Rmd, 0, NI), op=MIN)
    tt(fl(MX1, 0, NI), fl(Rmn, 0, NI), fl(Rmd, 0, NI), op=MAX)
    tt(fl(TF, 0, NI), fl(MX1, 0, NI), fl(Rmx, 0, NI), op=MIN)
    # boundary med3 staging (tiny)
    tt(bv(B1), bv(BA), bv(BB), op=MIN)
    tt(bv(B2), bv(BA), bv(BB), op=MAX)
    tt(bv(B3), bv(B2), bv(BC), op=MIN)

    def final(s0, s1_):
        ov = _seg(OUT, P, NSEG)[:, s0:s1_, 2:512]
        tt(ov, _seg(MN1, P, NSEG)[:, s0:s1_, 0:510],
           _seg(TF, P, NSEG)[:, s0:s1_, 0:510], op=MAX)
        tt(_seg(OUT, P, NSEG)[:, s0:s1_, 1:513:511],
           bv(B1)[:, s0:s1_], bv(B3)[:, s0:s1_], op=MAX)

    if stores is None:
        final(0, NSEG)
    else:
        # 4-way split: each chunk's store streams while the next chunk
        # computes, so only ~400KB remains to drain after the last op
        hg = GIMG // 2
        for s0, s1_ in ((0, hg), (hg, GIMG), (GIMG, GIMG + hg),
                        (GIMG + hg, NSEG)):
            final(s0, s1_)
            stores(s0, s1_)


def _block(nc, pio, pm, xh, oh, g, half, first=False, last=False):
    """One vertical half of one image group: odd output rows r0+1..r0+255,
    even rows r0+2..r0+256 (halves overlap by two rows so every DMA is a
    full 128-partition transfer). Rows 0 and 511 via the edge pass.
    first=True: loads and stage 1 ramp in 1+2+3 image chunks (cold start).
    last=True: stores go to the by-then-idle HWDGE queues so the final
    drain overlaps the edge pass."""
    r0 = 0 if half == 0 else H - HH - 2
    i0 = GIMG * g

    E = pio.tile([P, FP], BF16, tag="E", name="E")
    O = pio.tile([P, FP], BF16, tag="O", name="O")
    E_sh = pio.tile([P, FP], BF16, tag="E_sh", name="E_sh")
    O_sh2 = pio.tile([P, FP], BF16, tag="O_sh2", name="O_sh2")

    # scratch cols 512..513 of each segment are read by the flat stage-1
    # ops but never loaded: define them so no lane is uninitialized
    for T in (E, O, E_sh, O_sh2):
        nc.gpsimd.memset(_seg(T, P, GIMG)[:, :, W:PW], 0.0)

    def loads(ia, ib):
        lv = lambda T: _seg(T, P, GIMG)[:, ia:ib, 0:W]
        im = lambda r_lo: xh[
            r_lo : min(r_lo + 2 * P, H) : 2, i0 + ia : i0 + ib, :
        ]
        # queue order matters (HWDGE queues are FIFOs): the (O, E_sh) pair
        # feeds the first op of the block, so those loads go first
        nc.sync.dma_start(lv(E_sh), im(r0 + 2))     # rows r0+2p+2
        nc.scalar.dma_start(lv(O), im(r0 + 1))      # rows r0+2p+1
        nc.sync.dma_start(lv(E), im(r0))            # rows r0+2p
        nc.scalar.dma_start(lv(O_sh2), im(r0 + 3))  # rows r0+2p+3

    # stage-1 temps live in halves of stage-2 slots that are written later
    qpair = pm.tile([P, FP2], BF16, tag="Pmx", name="qpair")
    tpair = pm.tile([P, FP2], BF16, tag="Qmx", name="tpair")
    MN2 = pm.tile([P, FP2], BF16, tag="fMN", name="fMN")
    MD2 = pm.tile([P, FP2], BF16, tag="fMD", name="fMD")
    MX2 = pm.tile([P, FP2], BF16, tag="fMX", name="fMX")

    tt = nc.vector.tensor_tensor

    def stage1(ia, ib):
        a, b = ia * PW, ib * PW
        qv = lambda h: qpair[:][:, h * FP + a : h * FP + b]
        tv = lambda h: tpair[:][:, h * FP + a : h * FP + b]
        f = lambda T, h: T[:][:, h * FP + a : h * FP + b]
        sv = lambda T: T[:][:, a:b]
        qmn, qmx = qv(0), qv(1)
        # shared pair = (O, E_sh) = rows (2p+1, 2p+2)
        tt(qmn, sv(O), sv(E_sh), op=MIN)
        tt(qmx, sv(O), sv(E_sh), op=MAX)
        # field completion order MN, MX, MD matches the ACT copy order in
        # _stage2_fused so no s1 copy ever stalls the DVE.
        # odd output rows r0+2p+1: pair + E; even rows: pair + O_sh2
        tt(f(MN2, 0), qmn, sv(E), op=MIN)
        tt(f(MN2, 1), qmn, sv(O_sh2), op=MIN)
        tt(f(MX2, 0), qmx, sv(E), op=MAX)
        tt(f(MX2, 1), qmx, sv(O_sh2), op=MAX)
        tt(tv(0), qmx, sv(E), op=MIN)
        tt(f(MD2, 0), qmn, tv(0), op=MAX)
        tt(tv(1), qmx, sv(O_sh2), op=MIN)
        tt(f(MD2, 1), qmn, tv(1), op=MAX)

    if first:
        for ia, ib in ((0, 1), (1, 3), (3, 6)):
            loads(ia, ib)
            stage1(ia, ib)
    else:
        loads(0, GIMG)
        stage1(0, GIMG)

    OUT = pio.tile([P, FP2], BF16, tag="OUT", name="OUT")
    out_img = lambda r_lo: oh[r_lo : min(r_lo + 2 * P, H) : 2, i0 : i0 + GIMG, :]
    hi = GIMG // 2
    osv = _seg(OUT, P, NSEG)
    if last:
        # the end-of-kernel critical path is (last stores + HBM write ack):
        # stream each quarter of the output out the moment its final op
        # completes, rotating across the three DGE queues (the 16 SDMA
        # engines are shared, so this is about starting early, not BW)
        queues = [nc.gpsimd, nc.sync, nc.scalar, nc.gpsimd]

        def store_chunk(s0, s1_):
            r_lo = (r0 + 1) if s0 < GIMG else (r0 + 2)
            ia, ib = s0 % GIMG, (s1_ - 1) % GIMG + 1
            dst = oh[
                r_lo : min(r_lo + 2 * P, H) : 2, i0 + ia : i0 + ib, :
            ]
            queues[0].dma_start(dst, osv[:, s0:s1_, 1:513])
            queues.pop(0)

        _stage2_fused(nc, pm, MN2, MD2, MX2, OUT, stores=store_chunk)
    else:
        _stage2_fused(nc, pm, MN2, MD2, MX2, OUT)
        # stores on the SWDGE queue so they never block later blocks' loads
        nc.gpsimd.dma_start(out_img(r0 + 1), osv[:, 0:GIMG, 1:513])
        nc.gpsimd.dma_start(out_img(r0 + 2), osv[:, GIMG:NSEG, 1:513])


EP = 96   # edge partitions: p = chunk*24 + (edge*12 + img), 4 col-chunks
EW = 130  # edge grid width; position j of chunk c holds col 128c-1+j


def _edge_loads(nc, pio, xi):
    """Loads for image rows 0 and 511, reshaped to [96, 130]: each of the
    24 (edge,img) rows is split into 4 column-chunks of 128 with a 1-col
    halo on each side; the image-boundary halo positions are zeroed, so
    the pad columns fold into the grid and the edge compute needs NO
    boundary special-casing. Issued up front (tiny) so the end-of-kernel
    edge compute never waits on DMA."""
    R0 = pio.tile([EP, EW], BF16, tag="R0", name="R0")
    R1 = pio.tile([EP, EW], BF16, tag="R1", name="R1")
    for T in (R0, R1):
        # zero both halo columns on all partitions (the verifier rejects
        # partition ranges not starting at 0); loads then overwrite the
        # non-pad ones, leaving zeros only at chunk 0 col -1 / chunk 3
        # col 512
        nc.gpsimd.memset(T[:][0:EP, 0:1], 0.0)
        nc.gpsimd.memset(T[:][0:EP, 129:130], 0.0)
    for T, rows in ((R0, (0, H - 1)), (R1, (1, H - 2))):
        for k, r in enumerate(rows):  # k=0: slots 0..11, k=1: slots 12..23
            q = nc.sync if k == 0 else nc.scalar
            for c in range(4):
                p0 = 24 * c + 12 * k
                a, b = (1, 130) if c == 0 else (0, 129) if c == 3 else (0, 130)
                cols = slice(128 * c - 1 + a, 128 * c - 1 + b)
                q.dma_start(T[:][p0 : p0 + 12, a:b], xi[:, r, cols])
    return R0, R1


def _edge_compute(nc, pio, pm, oi, R0, R1):
    """Rows 0 and 511 (windows contain the zero pad row). Runs last, in
    the shadow of the final block's output stores."""
    NI = EW - 2  # 128

    def t2(tag):
        return pm.tile([EP, EW], BF16, tag=f"e{tag}", name=f"e{tag}")

    rmn, rmx = t2("rmn"), t2("rmx")
    tt = nc.vector.tensor_tensor
    tt(rmn[:], R0[:], R1[:], op=MIN)
    tt(rmx[:], R0[:], R1[:], op=MAX)

    # vertical sort3 with the zero pad row: min/max vs 0, med=max(mn,min(mx,0))
    MN0, MD0, MX0 = t2("MN"), t2("MD"), t2("MX")
    nc.vector.tensor_scalar_min(MN0[:], rmn[:], 0.0)
    nc.vector.tensor_scalar_max(MX0[:], rmx[:], 0.0)
    nc.vector.scalar_tensor_tensor(MD0[:], rmx[:], 0.0, rmn[:], op0=MIN, op1=MAX)

    # shifts on the DVE itself (2x_2P copies, ~0.2us each): the edge pass
    # must not touch the scalar engine, whose queue holds the LAST block's
    # loads right behind these instructions
    s1 = {}
    for name, F in (("MN", MN0), ("MD", MD0), ("MX", MX0)):
        T = t2(f"s1{name}")
        nc.vector.tensor_copy(T[:][0:EP, 0 : EW - 1], F[:][0:EP, 1:EW])
        s1[name] = T

    fl = lambda T, a, b: T[:][0:EP, a:b]
    Pmn, Pmx, Qmn, Qmx = t2("Pmn"), t2("Pmx"), t2("Qmn"), t2("Qmx")
    tmd, Rmn, Rmd, Rmx = t2("tmd"), t2("Rmn"), t2("Rmd"), t2("Rmx")
    MN1, MX1, TF = t2("MN1"), t2("MX1"), t2("TF")
    OUT0 = pio.tile([EP, EW], BF16, tag="OUT0", name="OUT0")

    tt(fl(Pmn, 0, NI), fl(MN0, 0, NI), fl(s1["MN"], 0, NI), op=MAX)
    tt(fl(Qmn, 0, NI), fl(MD0, 0, NI), fl(s1["MD"], 0, NI), op=MIN)
    tt(fl(Qmx, 0, NI), fl(MD0, 0, NI), fl(s1["MD"], 0, NI), op=MAX)
    tt(fl(Pmx, 0, NI), fl(MX0, 0, NI), fl(s1["MX"], 0, NI), op=MIN)
    tt(fl(Rmn, 0, NI), fl(Pmn, 0, NI), fl(MN0, 2, EW), op=MAX)
    tt(fl(tmd, 0, NI), fl(Qmx, 0, NI), fl(MD0, 2, EW), op=MIN)
    tt(fl(Rmd, 0, NI), fl(Qmn, 0, NI), fl(tmd, 0, NI), op=MAX)
    tt(fl(Rmx, 0, NI), fl(Pmx, 0, NI), fl(MX0, 2, EW), op=MIN)
    tt(fl(MN1, 0, NI), fl(Rmn, 0, NI), fl(Rmd, 0, NI), op=MIN)
    tt(fl(MX1, 0, NI), fl(Rmn, 0, NI), fl(Rmd, 0, NI), op=MAX)
    tt(fl(TF, 0, NI), fl(MX1, 0, NI), fl(Rmx, 0, NI), op=MIN)
    tt(fl(OUT0, 2, EW), fl(MN1, 0, NI), fl(TF, 0, NI), op=MAX)

    # SWDGE is empty by now (its last work was the mid-kernel stores), so
    # these tiny final stores' completion overlaps the HWDGE stores' ack
    for k, r in enumerate((0, H - 1)):
        for c in range(4):
            p0 = 24 * c + 12 * k
            nc.gpsimd.dma_start(
                oi[:, r, 128 * c : 128 * c + 128],
                OUT0[:][p0 : p0 + 12, 2:EW],
            )


def build_program():
    nc = bacc.Bacc(
        "TRN2", target_bir_lowering=False, debug=False, num_devices=N_CORES
    )
    x_d = nc.dram_tensor("x", [B_PER, C, H, W], BF16, kind="ExternalInput").ap()
    o_d = nc.dram_tensor("out", [B_PER, C, H, W], BF16, kind="ExternalOutput").ap()
    xh = x_d.rearrange("b c h w -> h (b c) w")  # [512, 12, 512]
    oh = o_d.rearrange("b c h w -> h (b c) w")
    xi = x_d.rearrange("b c h w -> (b c) h w")  # [12, 512, 512]
    oi = o_d.rearrange("b c h w -> (b c) h w")

    with tile.TileContext(nc) as tc:
        with (
            tc.tile_pool(name="io", bufs=1) as pio,
            tc.tile_pool(name="mid", bufs=1) as pm,
        ):
            nb = 2 * (NIMG // GIMG)
            _block(nc, pio, pm, xh, oh, 0, 0, first=True)
            R0, R1 = _edge_loads(nc, pio, xi)
            for i in range(1, nb - 1):
                _block(nc, pio, pm, xh, oh, i // 2, i % 2)
            # edge pass second-to-last: its stores drain under the final
            # block's compute, leaving only that block's (split) stores +
            # HBM write-ack on the end-of-kernel critical path
            _edge_compute(nc, pio, pm, oi, R0, R1)
            _block(nc, pio, pm, xh, oh, (nb - 1) // 2, (nb - 1) % 2,
                   last=True)
    nc.compile()
    return nc


def _get_program():
    global _PROGRAM
    if _PROGRAM is None:
        _PROGRAM = build_program()
    return _PROGRAM


def make_in_maps(x: np.ndarray):
    xb = np.ascontiguousarray(x).astype(ml_dtypes.bfloat16)
    return [{"x": xb[k * B_PER : (k + 1) * B_PER]} for k in range(N_CORES)]


def kernel(**inputs) -> np.ndarray:
    x = np.asarray(inputs["x"], dtype=np.float32)
    assert x.shape == (B, C, H, W), x.shape
    nc = _get_program()
    res = bass_utils.run_bass_kernel_spmd(
        nc, make_in_maps(x), core_ids=list(range(N_CORES))
    )
    out = np.concatenate(
        [np.asarray(res.results[k]["out"]) for k in range(N_CORES)], axis=0
    )
    return out.astype(np.float32)


# revision 33
# speedup vs baseline: 1.0050x; 1.0017x over previous
"""3x3 median filter (zero-padded) on TRN2, 8 NeuronCores, bf16 datapath.

Input  x: (32, 3, 512, 512) float32
Output  : (32, 3, 512, 512) float32.

Accuracy: the median network only ever SELECTS one of its 9 inputs (min/max
ops create no new values), so the device-side bf16 result equals the bf16
rounding of the element that is the median of the rounded window. Order
statistics are 1-Lipschitz under sup-norm perturbation, so end-to-end error
is <= 2^-8 relative -- far inside the 2e-2 gate. Measured 3.4e-3.

Strategy
--------
Pure data parallel: batch dim sharded 4-per-core across 8 cores; per core
12 images (4 batch x 3 chan) in 2 groups of 6 images x 2 vertical halves.

bf16 doubles DVE tensor_tensor throughput (2x_1P perf mode) but ONLY for
unit-stride 4-byte-aligned access patterns, so the horizontal stage is
restructured from the fp32 baseline's stride-2 parity tricks into dense
shifted-field form. Per field F in {MN,MD,MX}: s1F[j]=F[j+1] is the ONLY
odd-element shift (a ScalarE copy -- ACT is otherwise idle), then
   P[j] = op(F[j], s1F[j])          # aligned TT, 2x
   R[j] = op(P[j], F[j+2])          # +2 elems = 4B-aligned shift, 2x
R[j] = sliding-3 result centered at col j+1; the final med3(Rmn,Rmd,Rmx)
writes into an OUT grid whose per-image segment holds col c at position
c+1, so the wide write starts at even offset 2 and the DMA store (which
doesn't care about alignment) un-shifts.

Both row parities' fields live in ONE fused [128, 12*514] tile (odd-parity
images = segments 0..5, even = 6..11), so stage 2 is 12 double-width ops
per block instead of 24 -- per-op overhead (58-cycle issue + ~90ns DRAIN)
is the only thing that changes, the streamed cycles are identical. Output
cols 0 and 511 (windows containing the zero pad column) are 7 tiny ops on
gathered grid positions {0,510} across all 12 segments.

Grid: per-image segment width 514 (even -> every segment start keeps 4B
parity). Segment positions 512..513 are scratch: stage-1 ops run flat over
the whole grid and compute garbage there; no stored output reads a garbage
lane (out cols 0/511 come from the boundary path).

SBUF fits via aggressive aliasing with DVE-program-order-safe lifetimes:
stage-1's qmn/qmx/t_o/t_e live in halves of stage-2's Pmx/Qmx slots;
stage-2's Rmx/tmd/Rmd overwrite the dead MN/MX/MD field buffers; MN1/MX1/TF
overwrite Pmn/Qmn/Qmx. Stage-1 emits fields in MN,MX,MD order and ACT
copies s1MN,s1MX,s1MD in that order so every copy lands before the DVE
needs it, with no stall.

Image rows 0 and 511 (windows contain the zero pad row): one small
24-partition pass issued LAST so it fills the DVE-idle tail while the
final block's output stores (sent to the idle HWDGE queues) drain. Its
tiny loads are issued up front. Block 0's loads+stage-1 ramp up in 1+2+3
image chunks sized to the ~250GB/s strided-row-gather DMA rate, so the
DVE starts ~2us after the first 0.25MB lands instead of waiting for 3MB.

Engine budget per core: DVE 17 TT/elem at 2x ~= 236us busy (the floor for
this decomposition), ACT ~75us, DMA ~19MB. Loads on the SP+ACT HWDGE
queues, mid-kernel stores on the GpSimd SWDGE queue.
"""
import sys

if "/opt/trn_rl_repo" not in sys.path:
    sys.path.insert(0, "/opt/trn_rl_repo")

import numpy as np
import ml_dtypes
import concourse.bacc as bacc
import concourse.mybir as mybir
import concourse.tile as tile
from concourse import bass_utils

B, C, H, W = 32, 3, 512, 512
N_CORES = 8
B_PER = B // N_CORES          # 4 batches per core
NIMG = B_PER * C              # 12 images per core
GIMG = 6                      # images per tile group
PW = W + 2                    # per-image grid width (514, even)
FP = GIMG * PW                # half (one parity) grid width (3084)
FP2 = 2 * FP                  # fused two-parity grid width (6168)
NSEG = 2 * GIMG               # segments in a fused tile (12)
HH = H // 2                   # 256 rows per vertical half
P = 128                       # partitions = row pairs per half
NE = 2 * NIMG                 # partitions in the edge-rows pass (24)

BF16 = mybir.dt.bfloat16
MIN = mybir.AluOpType.min
MAX = mybir.AluOpType.max

_PROGRAM = None


def _seg(T, npart, nseg):
    """[npart, nseg, 514] per-image-segment view."""
    return T[:].rearrange("p (i w) -> p i w", w=PW)[0:npart, 0:nseg]


def _stage2_fused(nc, pm, MN, MD, MX, OUT, stores=None):
    """Both parities of stage 2 in double-width ops on the fused grids.
    ScalarE does the three s1 shifts (issued in MN,MX,MD order to match
    the DVE's consumption order). Aliases overwrite only dead buffers.
    stores: optional per-parity store callbacks; when given, the final op
    and boundary write are split per parity and each parity's store is
    issued the moment its OUT half is complete (used by the last block to
    shrink the end-of-kernel store drain)."""
    NI = FP2 - 2
    fl = lambda T, a, b: T[:][:, a:b]

    s1 = {}
    for nm, F in (("MN", MN), ("MX", MX), ("MD", MD)):
        T = pm.tile([P, FP2], BF16, tag=f"s1{nm}", name=f"s1{nm}")
        nc.scalar.copy(T[:][:, 0 : FP2 - 1], F[:][:, 1:FP2])
        s1[nm] = T

    def t2(tag):
        return pm.tile([P, FP2], BF16, tag=tag, name=tag)

    def alias(tag, name):
        return pm.tile([P, FP2], BF16, tag=tag, name=name)

    Pmn, Pmx, Qmn, Qmx, Rmn = t2("Pmn"), t2("Pmx"), t2("Qmn"), t2("Qmx"), t2("Rmn")
    Rmx = alias("fMN", "Rmx")   # MN field dead after Rmn
    tmd = alias("fMX", "tmd")   # MX field dead after Rmx
    Rmd = alias("fMD", "Rmd")   # MD field dead after tmd
    MN1 = alias("Pmn", "MN1")
    MX1 = alias("Qmn", "MX1")
    TF = alias("Qmx", "TF")

    tt = nc.vector.tensor_tensor
    gv = lambda T: _seg(T, P, NSEG)[:, :, 0:511:510]
    bt = lambda tag: pm.tile([P, 2 * NSEG], BF16, tag=f"{tag}b", name=f"{tag}b")
    bv = lambda T: T[:].rearrange("p (i c) -> p i c", c=2)[0:P, 0:NSEG]
    BA, BB, BC = bt("BA"), bt("BB"), bt("BC")
    B1, B2, B3 = bt("B1"), bt("B2"), bt("B3")

    tt(fl(Pmn, 0, NI), fl(MN, 0, NI), fl(s1["MN"], 0, NI), op=MAX)
    tt(fl(Pmx, 0, NI), fl(MX, 0, NI), fl(s1["MX"], 0, NI), op=MIN)
    nc.vector.tensor_scalar_max(bv(BA), gv(Pmn), 0.0)
    nc.vector.tensor_scalar_min(bv(BC), gv(Pmx), 0.0)
    tt(fl(Rmn, 0, NI), fl(Pmn, 0, NI), fl(MN, 2, FP2), op=MAX)
    tt(fl(Rmx, 0, NI), fl(Pmx, 0, NI), fl(MX, 2, FP2), op=MIN)
    tt(fl(Qmn, 0, NI), fl(MD, 0, NI), fl(s1["MD"], 0, NI), op=MIN)
    tt(fl(Qmx, 0, NI), fl(MD, 0, NI), fl(s1["MD"], 0, NI), op=MAX)
    nc.vector.scalar_tensor_tensor(bv(BB), gv(Qmx), 0.0, gv(Qmn), op0=MIN, op1=MAX)
    tt(fl(tmd, 0, NI), fl(Qmx, 0, NI), fl(MD, 2, FP2), op=MIN)
    tt(fl(Rmd, 0, NI), fl(Qmn, 0, NI), fl(tmd, 0, NI), op=MAX)
    # med3(Rmn, Rmd, Rmx); last wide op writes output cols 1..510 at grid
    # positions 2..511 (even start -> stays in 2x mode)
    # boundary med3 staging (tiny; needed by every chunk's boundary write)
    tt(bv(B1), bv(BA), bv(BB), op=MIN)
    tt(bv(B2), bv(BA), bv(BB), op=MAX)
    tt(bv(B3), bv(B2), bv(BC), op=MIN)

    def tail(s0, s1_):
        a, b = s0 * PW, min(s1_ * PW, NI)
        tt(fl(MN1, a, b), fl(Rmn, a, b), fl(Rmd, a, b), op=MIN)
        tt(fl(MX1, a, b), fl(Rmn, a, b), fl(Rmd, a, b), op=MAX)
        tt(fl(TF, a, b), fl(MX1, a, b), fl(Rmx, a, b), op=MIN)
        ov = _seg(OUT, P, NSEG)[:, s0:s1_, 2:512]
        tt(ov, _seg(MN1, P, NSEG)[:, s0:s1_, 0:510],
           _seg(TF, P, NSEG)[:, s0:s1_, 0:510], op=MAX)
        tt(_seg(OUT, P, NSEG)[:, s0:s1_, 1:513:511],
           bv(B1)[:, s0:s1_], bv(B3)[:, s0:s1_], op=MAX)

    if stores is None:
        tail(0, NSEG)
    else:
        # 4-way split of the whole tail chain: quarter q's store streams
        # while quarter q+1 computes, so the first store starts ~7us
        # earlier and only ~400KB remains to drain after the last op
        hg = GIMG // 2
        for s0, s1_ in ((0, hg), (hg, GIMG), (GIMG, GIMG + hg),
                        (GIMG + hg, NSEG)):
            tail(s0, s1_)
            stores(s0, s1_)


def _block(nc, pio, pm, xh, oh, g, half, first=False, last=False):
    """One vertical half of one image group: odd output rows r0+1..r0+255,
    even rows r0+2..r0+256 (halves overlap by two rows so every DMA is a
    full 128-partition transfer). Rows 0 and 511 via the edge pass.
    first=True: loads and stage 1 ramp in 1+2+3 image chunks (cold start).
    last=True: stores go to the by-then-idle HWDGE queues so the final
    drain overlaps the edge pass."""
    r0 = 0 if half == 0 else H - HH - 2
    i0 = GIMG * g

    E = pio.tile([P, FP], BF16, tag="E", name="E")
    O = pio.tile([P, FP], BF16, tag="O", name="O")
    E_sh = pio.tile([P, FP], BF16, tag="E_sh", name="E_sh")
    O_sh2 = pio.tile([P, FP], BF16, tag="O_sh2", name="O_sh2")

    # scratch cols 512..513 of each segment are read by the flat stage-1
    # ops but never loaded: define them so no lane is uninitialized
    for T in (E, O, E_sh, O_sh2):
        nc.gpsimd.memset(_seg(T, P, GIMG)[:, :, W:PW], 0.0)

    def loads(ia, ib):
        lv = lambda T: _seg(T, P, GIMG)[:, ia:ib, 0:W]
        im = lambda r_lo: xh[
            r_lo : min(r_lo + 2 * P, H) : 2, i0 + ia : i0 + ib, :
        ]
        # queue order matters (HWDGE queues are FIFOs): the (O, E_sh) pair
        # feeds the first op of the block, so those loads go first
        nc.sync.dma_start(lv(E_sh), im(r0 + 2))     # rows r0+2p+2
        nc.scalar.dma_start(lv(O), im(r0 + 1))      # rows r0+2p+1
        nc.sync.dma_start(lv(E), im(r0))            # rows r0+2p
        nc.scalar.dma_start(lv(O_sh2), im(r0 + 3))  # rows r0+2p+3

    # stage-1 temps live in halves of stage-2 slots that are written later
    qpair = pm.tile([P, FP2], BF16, tag="Pmx", name="qpair")
    tpair = pm.tile([P, FP2], BF16, tag="Qmx", name="tpair")
    MN2 = pm.tile([P, FP2], BF16, tag="fMN", name="fMN")
    MD2 = pm.tile([P, FP2], BF16, tag="fMD", name="fMD")
    MX2 = pm.tile([P, FP2], BF16, tag="fMX", name="fMX")

    tt = nc.vector.tensor_tensor

    def stage1(ia, ib):
        a, b = ia * PW, ib * PW
        qv = lambda h: qpair[:][:, h * FP + a : h * FP + b]
        tv = lambda h: tpair[:][:, h * FP + a : h * FP + b]
        f = lambda T, h: T[:][:, h * FP + a : h * FP + b]
        sv = lambda T: T[:][:, a:b]
        qmn, qmx = qv(0), qv(1)
        # shared pair = (O, E_sh) = rows (2p+1, 2p+2)
        tt(qmn, sv(O), sv(E_sh), op=MIN)
        tt(qmx, sv(O), sv(E_sh), op=MAX)
        # field completion order MN, MX, MD matches the ACT copy order in
        # _stage2_fused so no s1 copy ever stalls the DVE.
        # odd output rows r0+2p+1: pair + E; even rows: pair + O_sh2
        tt(f(MN2, 0), qmn, sv(E), op=MIN)
        tt(f(MN2, 1), qmn, sv(O_sh2), op=MIN)
        tt(f(MX2, 0), qmx, sv(E), op=MAX)
        tt(f(MX2, 1), qmx, sv(O_sh2), op=MAX)
        tt(tv(0), qmx, sv(E), op=MIN)
        tt(f(MD2, 0), qmn, tv(0), op=MAX)
        tt(tv(1), qmx, sv(O_sh2), op=MIN)
        tt(f(MD2, 1), qmn, tv(1), op=MAX)

    if first:
        for ia, ib in ((0, 1), (1, 3), (3, 6)):
            loads(ia, ib)
            stage1(ia, ib)
    else:
        loads(0, GIMG)
        stage1(0, GIMG)

    OUT = pio.tile([P, FP2], BF16, tag="OUT", name="OUT")
    out_img = lambda r_lo: oh[r_lo : min(r_lo + 2 * P, H) : 2, i0 : i0 + GIMG, :]
    hi = GIMG // 2
    osv = _seg(OUT, P, NSEG)
    if last:
        # the end-of-kernel critical path is (last stores + HBM write ack):
        # stream each quarter of the output out the moment its final op
        # completes, rotating across the three DGE queues (the 16 SDMA
        # engines are shared, so this is about starting early, not BW)
        # first and last chunks on HWDGE (fast ~0.6us first-byte; both
        # queues are empty by now): first starts the drain ASAP, last is
        # the end-of-kernel critical path
        queues = [nc.scalar, nc.gpsimd, nc.gpsimd, nc.sync]

        def store_chunk(s0, s1_):
            r_lo = (r0 + 1) if s0 < GIMG else (r0 + 2)
            ia, ib = s0 % GIMG, (s1_ - 1) % GIMG + 1
            dst = oh[
                r_lo : min(r_lo + 2 * P, H) : 2, i0 + ia : i0 + ib, :
            ]
            queues[0].dma_start(dst, osv[:, s0:s1_, 1:513])
            queues.pop(0)

        _stage2_fused(nc, pm, MN2, MD2, MX2, OUT, stores=store_chunk)
    else:
        _stage2_fused(nc, pm, MN2, MD2, MX2, OUT)
        # stores on the SWDGE queue so they never block later blocks' loads
        nc.gpsimd.dma_start(out_img(r0 + 1), osv[:, 0:GIMG, 1:513])
        nc.gpsimd.dma_start(out_img(r0 + 2), osv[:, GIMG:NSEG, 1:513])


EP = 96   # edge partitions: p = chunk*24 + (edge*12 + img), 4 col-chunks
EW = 130  # edge grid width; position j of chunk c holds col 128c-1+j


def _edge_loads(nc, pio, xi):
    """Loads for image rows 0 and 511, reshaped to [96, 130]: each of the
    24 (edge,img) rows is split into 4 column-chunks of 128 with a 1-col
    halo on each side; the image-boundary halo positions are zeroed, so
    the pad columns fold into the grid and the edge compute needs NO
    boundary special-casing. Issued up front (tiny) so the end-of-kernel
    edge compute never waits on DMA."""
    R0 = pio.tile([EP, EW], BF16, tag="R0", name="R0")
    R1 = pio.tile([EP, EW], BF16, tag="R1", name="R1")
    for T in (R0, R1):
        # zero both halo columns on all partitions (the verifier rejects
        # partition ranges not starting at 0); loads then overwrite the
        # non-pad ones, leaving zeros only at chunk 0 col -1 / chunk 3
        # col 512
        nc.gpsimd.memset(T[:][0:EP, 0:1], 0.0)
        nc.gpsimd.memset(T[:][0:EP, 129:130], 0.0)
    for T, rows in ((R0, (0, H - 1)), (R1, (1, H - 2))):
        for k, r in enumerate(rows):  # k=0: slots 0..11, k=1: slots 12..23
            q = nc.sync if k == 0 else nc.scalar
            for c in range(4):
                p0 = 24 * c + 12 * k
                a, b = (1, 130) if c == 0 else (0, 129) if c == 3 else (0, 130)
                cols = slice(128 * c - 1 + a, 128 * c - 1 + b)
                q.dma_start(T[:][p0 : p0 + 12, a:b], xi[:, r, cols])
    return R0, R1


def _edge_compute(nc, pio, pm, oi, R0, R1):
    """Rows 0 and 511 (windows contain the zero pad row). Runs last, in
    the shadow of the final block's output stores."""
    NI = EW - 2  # 128

    def t2(tag):
        return pm.tile([EP, EW], BF16, tag=f"e{tag}", name=f"e{tag}")

    rmn, rmx = t2("rmn"), t2("rmx")
    tt = nc.vector.tensor_tensor
    tt(rmn[:], R0[:], R1[:], op=MIN)
    tt(rmx[:], R0[:], R1[:], op=MAX)

    # vertical sort3 with the zero pad row: min/max vs 0, med=max(mn,min(mx,0))
    MN0, MD0, MX0 = t2("MN"), t2("MD"), t2("MX")
    nc.vector.tensor_scalar_min(MN0[:], rmn[:], 0.0)
    nc.vector.tensor_scalar_max(MX0[:], rmx[:], 0.0)
    nc.vector.scalar_tensor_tensor(MD0[:], rmx[:], 0.0, rmn[:], op0=MIN, op1=MAX)

    # shifts on the DVE itself (2x_2P copies, ~0.2us each): the edge pass
    # must not touch the scalar engine, whose queue holds the LAST block's
    # loads right behind these instructions
    s1 = {}
    for name, F in (("MN", MN0), ("MD", MD0), ("MX", MX0)):
        T = t2(f"s1{name}")
        nc.vector.tensor_copy(T[:][0:EP, 0 : EW - 1], F[:][0:EP, 1:EW])
        s1[name] = T

    fl = lambda T, a, b: T[:][0:EP, a:b]
    Pmn, Pmx, Qmn, Qmx = t2("Pmn"), t2("Pmx"), t2("Qmn"), t2("Qmx")
    tmd, Rmn, Rmd, Rmx = t2("tmd"), t2("Rmn"), t2("Rmd"), t2("Rmx")
    MN1, MX1, TF = t2("MN1"), t2("MX1"), t2("TF")
    OUT0 = pio.tile([EP, EW], BF16, tag="OUT0", name="OUT0")

    tt(fl(Pmn, 0, NI), fl(MN0, 0, NI), fl(s1["MN"], 0, NI), op=MAX)
    tt(fl(Qmn, 0, NI), fl(MD0, 0, NI), fl(s1["MD"], 0, NI), op=MIN)
    tt(fl(Qmx, 0, NI), fl(MD0, 0, NI), fl(s1["MD"], 0, NI), op=MAX)
    tt(fl(Pmx, 0, NI), fl(MX0, 0, NI), fl(s1["MX"], 0, NI), op=MIN)
    tt(fl(Rmn, 0, NI), fl(Pmn, 0, NI), fl(MN0, 2, EW), op=MAX)
    tt(fl(tmd, 0, NI), fl(Qmx, 0, NI), fl(MD0, 2, EW), op=MIN)
    tt(fl(Rmd, 0, NI), fl(Qmn, 0, NI), fl(tmd, 0, NI), op=MAX)
    tt(fl(Rmx, 0, NI), fl(Pmx, 0, NI), fl(MX0, 2, EW), op=MIN)
    tt(fl(MN1, 0, NI), fl(Rmn, 0, NI), fl(Rmd, 0, NI), op=MIN)
    tt(fl(MX1, 0, NI), fl(Rmn, 0, NI), fl(Rmd, 0, NI), op=MAX)
    tt(fl(TF, 0, NI), fl(MX1, 0, NI), fl(Rmx, 0, NI), op=MIN)
    tt(fl(OUT0, 2, EW), fl(MN1, 0, NI), fl(TF, 0, NI), op=MAX)

    # SWDGE is empty by now (its last work was the mid-kernel stores), so
    # these tiny final stores' completion overlaps the HWDGE stores' ack
    for k, r in enumerate((0, H - 1)):
        for c in range(4):
            p0 = 24 * c + 12 * k
            nc.gpsimd.dma_start(
                oi[:, r, 128 * c : 128 * c + 128],
                OUT0[:][p0 : p0 + 12, 2:EW],
            )


def build_program():
    nc = bacc.Bacc(
        "TRN2", target_bir_lowering=False, debug=False, num_devices=N_CORES
    )
    x_d = nc.dram_tensor("x", [B_PER, C, H, W], BF16, kind="ExternalInput").ap()
    o_d = nc.dram_tensor("out", [B_PER, C, H, W], BF16, kind="ExternalOutput").ap()
    xh = x_d.rearrange("b c h w -> h (b c) w")  # [512, 12, 512]
    oh = o_d.rearrange("b c h w -> h (b c) w")
    xi = x_d.rearrange("b c h w -> (b c) h w")  # [12, 512, 512]
    oi = o_d.rearrange("b c h w -> (b c) h w")

    with tile.TileContext(nc) as tc:
        with (
            tc.tile_pool(name="io", bufs=1) as pio,
            tc.tile_pool(name="mid", bufs=1) as pm,
        ):
            nb = 2 * (NIMG // GIMG)
            _block(nc, pio, pm, xh, oh, 0, 0, first=True)
            R0, R1 = _edge_loads(nc, pio, xi)
            for i in range(1, nb - 1):
                _block(nc, pio, pm, xh, oh, i // 2, i % 2)
            # edge pass second-to-last: its stores drain under the final
            # block's compute, leaving only that block's (split) stores +
            # HBM write-ack on the end-of-kernel critical path
            _edge_compute(nc, pio, pm, oi, R0, R1)
            _block(nc, pio, pm, xh, oh, (nb - 1) // 2, (nb - 1) % 2,
                   last=True)
    nc.compile()
    return nc


def _get_program():
    global _PROGRAM
    if _PROGRAM is None:
        _PROGRAM = build_program()
    return _PROGRAM


def make_in_maps(x: np.ndarray):
    xb = np.ascontiguousarray(x).astype(ml_dtypes.bfloat16)
    return [{"x": xb[k * B_PER : (k + 1) * B_PER]} for k in range(N_CORES)]


def kernel(**inputs) -> np.ndarray:
    x = np.asarray(inputs["x"], dtype=np.float32)
    assert x.shape == (B, C, H, W), x.shape
    nc = _get_program()
    res = bass_utils.run_bass_kernel_spmd(
        nc, make_in_maps(x), core_ids=list(range(N_CORES))
    )
    out = np.concatenate(
        [np.asarray(res.results[k]["out"]) for k in range(N_CORES)], axis=0
    )
    return out.astype(np.float32)


# revision 35
# speedup vs baseline: 1.0069x; 1.0019x over previous
"""3x3 median filter (zero-padded) on TRN2, 8 NeuronCores, bf16 datapath.

Input  x: (32, 3, 512, 512) float32
Output  : (32, 3, 512, 512) float32.

Accuracy: the median network only ever SELECTS one of its 9 inputs (min/max
ops create no new values), so the device-side bf16 result equals the bf16
rounding of the element that is the median of the rounded window. Order
statistics are 1-Lipschitz under sup-norm perturbation, so end-to-end error
is <= 2^-8 relative -- far inside the 2e-2 gate. Measured 3.4e-3.

Strategy
--------
Pure data parallel: batch dim sharded 4-per-core across 8 cores; per core
12 images (4 batch x 3 chan) in 2 groups of 6 images x 2 vertical halves.

bf16 doubles DVE tensor_tensor throughput (2x_1P perf mode) but ONLY for
unit-stride 4-byte-aligned access patterns, so the horizontal stage is
restructured from the fp32 baseline's stride-2 parity tricks into dense
shifted-field form. Per field F in {MN,MD,MX}: s1F[j]=F[j+1] is the ONLY
odd-element shift (a ScalarE copy -- ACT is otherwise idle), then
   P[j] = op(F[j], s1F[j])          # aligned TT, 2x
   R[j] = op(P[j], F[j+2])          # +2 elems = 4B-aligned shift, 2x
R[j] = sliding-3 result centered at col j+1; the final med3(Rmn,Rmd,Rmx)
writes into an OUT grid whose per-image segment holds col c at position
c+1, so the wide write starts at even offset 2 and the DMA store (which
doesn't care about alignment) un-shifts.

Both row parities' fields live in ONE fused [128, 12*514] tile (odd-parity
images = segments 0..5, even = 6..11), so stage 2 is 12 double-width ops
per block instead of 24 -- per-op overhead (58-cycle issue + ~90ns DRAIN)
is the only thing that changes, the streamed cycles are identical. Output
cols 0 and 511 (windows containing the zero pad column) are 7 tiny ops on
gathered grid positions {0,510} across all 12 segments.

Grid: per-image segment width 514 (even -> every segment start keeps 4B
parity). Segment positions 512..513 are scratch: stage-1 ops run flat over
the whole grid and compute garbage there; no stored output reads a garbage
lane (out cols 0/511 come from the boundary path).

SBUF fits via aggressive aliasing with DVE-program-order-safe lifetimes:
stage-1's qmn/qmx/t_o/t_e live in halves of stage-2's Pmx/Qmx slots;
stage-2's Rmx/tmd/Rmd overwrite the dead MN/MX/MD field buffers; MN1/MX1/TF
overwrite Pmn/Qmn/Qmx. Stage-1 emits fields in MN,MX,MD order and ACT
copies s1MN,s1MX,s1MD in that order so every copy lands before the DVE
needs it, with no stall.

Image rows 0 and 511 (windows contain the zero pad row): one small
24-partition pass issued LAST so it fills the DVE-idle tail while the
final block's output stores (sent to the idle HWDGE queues) drain. Its
tiny loads are issued up front. Block 0's loads+stage-1 ramp up in 1+2+3
image chunks sized to the ~250GB/s strided-row-gather DMA rate, so the
DVE starts ~2us after the first 0.25MB lands instead of waiting for 3MB.

Engine budget per core: DVE 17 TT/elem at 2x ~= 236us busy (the floor for
this decomposition), ACT ~75us, DMA ~19MB. Loads on the SP+ACT HWDGE
queues, mid-kernel stores on the GpSimd SWDGE queue.
"""
import sys

if "/opt/trn_rl_repo" not in sys.path:
    sys.path.insert(0, "/opt/trn_rl_repo")

import numpy as np
import ml_dtypes
import concourse.bacc as bacc
import concourse.mybir as mybir
import concourse.tile as tile
from concourse import bass_utils

B, C, H, W = 32, 3, 512, 512
N_CORES = 8
B_PER = B // N_CORES          # 4 batches per core
NIMG = B_PER * C              # 12 images per core
GIMG = 6                      # images per tile group
PW = W + 2                    # per-image grid width (514, even)
FP = GIMG * PW                # half (one parity) grid width (3084)
FP2 = 2 * FP                  # fused two-parity grid width (6168)
NSEG = 2 * GIMG               # segments in a fused tile (12)
HH = H // 2                   # 256 rows per vertical half
P = 128                       # partitions = row pairs per half
NE = 2 * NIMG                 # partitions in the edge-rows pass (24)

BF16 = mybir.dt.bfloat16
MIN = mybir.AluOpType.min
MAX = mybir.AluOpType.max

_PROGRAM = None


def _seg(T, npart, nseg):
    """[npart, nseg, 514] per-image-segment view."""
    return T[:].rearrange("p (i w) -> p i w", w=PW)[0:npart, 0:nseg]


def _stage2_fused(nc, pm, MN, MD, MX, OUT, stores=None):
    """Both parities of stage 2 in double-width ops on the fused grids.
    ScalarE does the three s1 shifts (issued in MN,MX,MD order to match
    the DVE's consumption order). Aliases overwrite only dead buffers.
    stores: optional per-parity store callbacks; when given, the final op
    and boundary write are split per parity and each parity's store is
    issued the moment its OUT half is complete (used by the last block to
    shrink the end-of-kernel store drain)."""
    NI = FP2 - 2
    fl = lambda T, a, b: T[:][:, a:b]

    s1 = {}
    for nm, F in (("MN", MN), ("MX", MX), ("MD", MD)):
        T = pm.tile([P, FP2], BF16, tag=f"s1{nm}", name=f"s1{nm}")
        nc.scalar.copy(T[:][:, 0 : FP2 - 1], F[:][:, 1:FP2])
        s1[nm] = T

    def t2(tag):
        return pm.tile([P, FP2], BF16, tag=tag, name=tag)

    def alias(tag, name):
        return pm.tile([P, FP2], BF16, tag=tag, name=name)

    Pmn, Pmx, Qmn, Qmx, Rmn = t2("Pmn"), t2("Pmx"), t2("Qmn"), t2("Qmx"), t2("Rmn")
    Rmx = alias("fMN", "Rmx")   # MN field dead after Rmn
    tmd = alias("fMX", "tmd")   # MX field dead after Rmx
    Rmd = alias("fMD", "Rmd")   # MD field dead after tmd
    MN1 = alias("Pmn", "MN1")
    MX1 = alias("Qmn", "MX1")
    TF = alias("Qmx", "TF")

    tt = nc.vector.tensor_tensor
    gv = lambda T: _seg(T, P, NSEG)[:, :, 0:511:510]
    bt = lambda tag: pm.tile([P, 2 * NSEG], BF16, tag=f"{tag}b", name=f"{tag}b")
    bv = lambda T: T[:].rearrange("p (i c) -> p i c", c=2)[0:P, 0:NSEG]
    BA, BB, BC = bt("BA"), bt("BB"), bt("BC")
    B1, B2, B3 = bt("B1"), bt("B2"), bt("B3")

    tt(fl(Pmn, 0, NI), fl(MN, 0, NI), fl(s1["MN"], 0, NI), op=MAX)
    tt(fl(Pmx, 0, NI), fl(MX, 0, NI), fl(s1["MX"], 0, NI), op=MIN)
    nc.vector.tensor_scalar_max(bv(BA), gv(Pmn), 0.0)
    nc.vector.tensor_scalar_min(bv(BC), gv(Pmx), 0.0)
    tt(fl(Rmn, 0, NI), fl(Pmn, 0, NI), fl(MN, 2, FP2), op=MAX)
    tt(fl(Rmx, 0, NI), fl(Pmx, 0, NI), fl(MX, 2, FP2), op=MIN)
    tt(fl(Qmn, 0, NI), fl(MD, 0, NI), fl(s1["MD"], 0, NI), op=MIN)
    tt(fl(Qmx, 0, NI), fl(MD, 0, NI), fl(s1["MD"], 0, NI), op=MAX)
    nc.vector.scalar_tensor_tensor(bv(BB), gv(Qmx), 0.0, gv(Qmn), op0=MIN, op1=MAX)
    tt(fl(tmd, 0, NI), fl(Qmx, 0, NI), fl(MD, 2, FP2), op=MIN)
    tt(fl(Rmd, 0, NI), fl(Qmn, 0, NI), fl(tmd, 0, NI), op=MAX)
    # med3(Rmn, Rmd, Rmx); last wide op writes output cols 1..510 at grid
    # positions 2..511 (even start -> stays in 2x mode)
    # boundary med3 staging (tiny; needed by every chunk's boundary write)
    tt(bv(B1), bv(BA), bv(BB), op=MIN)
    tt(bv(B2), bv(BA), bv(BB), op=MAX)
    tt(bv(B3), bv(B2), bv(BC), op=MIN)

    def tail(s0, s1_):
        a, b = s0 * PW, min(s1_ * PW, NI)
        tt(fl(MN1, a, b), fl(Rmn, a, b), fl(Rmd, a, b), op=MIN)
        tt(fl(MX1, a, b), fl(Rmn, a, b), fl(Rmd, a, b), op=MAX)
        tt(fl(TF, a, b), fl(MX1, a, b), fl(Rmx, a, b), op=MIN)
        ov = _seg(OUT, P, NSEG)[:, s0:s1_, 2:512]
        tt(ov, _seg(MN1, P, NSEG)[:, s0:s1_, 0:510],
           _seg(TF, P, NSEG)[:, s0:s1_, 0:510], op=MAX)
        tt(_seg(OUT, P, NSEG)[:, s0:s1_, 1:513:511],
           bv(B1)[:, s0:s1_], bv(B3)[:, s0:s1_], op=MAX)

    if stores is None:
        tail(0, NSEG)
    else:
        # split the whole tail chain: chunk q's store streams while chunk
        # q+1 computes; the LAST chunk is a single segment (~131KB) so the
        # end-of-kernel drain link is minimal (chunks may not cross the
        # parity boundary at segment 6 -- stores map to different rows)
        for s0, s1_ in ((0, 3), (3, GIMG), (GIMG, NSEG - 1),
                        (NSEG - 1, NSEG)):
            tail(s0, s1_)
            stores(s0, s1_)


def _block(nc, pio, pm, xh, oh, g, half, first=False, last=False):
    """One vertical half of one image group: odd output rows r0+1..r0+255,
    even rows r0+2..r0+256 (halves overlap by two rows so every DMA is a
    full 128-partition transfer). Rows 0 and 511 via the edge pass.
    first=True: loads and stage 1 ramp in 1+2+3 image chunks (cold start).
    last=True: stores go to the by-then-idle HWDGE queues so the final
    drain overlaps the edge pass."""
    r0 = 0 if half == 0 else H - HH - 2
    i0 = GIMG * g

    E = pio.tile([P, FP], BF16, tag="E", name="E")
    O = pio.tile([P, FP], BF16, tag="O", name="O")
    E_sh = pio.tile([P, FP], BF16, tag="E_sh", name="E_sh")
    O_sh2 = pio.tile([P, FP], BF16, tag="O_sh2", name="O_sh2")

    # scratch cols 512..513 of each segment are read by the flat stage-1
    # ops but never loaded: define them so no lane is uninitialized
    for T in (E, O, E_sh, O_sh2):
        nc.gpsimd.memset(_seg(T, P, GIMG)[:, :, W:PW], 0.0)

    def loads(ia, ib):
        lv = lambda T: _seg(T, P, GIMG)[:, ia:ib, 0:W]
        im = lambda r_lo: xh[
            r_lo : min(r_lo + 2 * P, H) : 2, i0 + ia : i0 + ib, :
        ]
        # queue order matters (HWDGE queues are FIFOs): the (O, E_sh) pair
        # feeds the first op of the block, so those loads go first
        nc.sync.dma_start(lv(E_sh), im(r0 + 2))     # rows r0+2p+2
        nc.scalar.dma_start(lv(O), im(r0 + 1))      # rows r0+2p+1
        nc.sync.dma_start(lv(E), im(r0))            # rows r0+2p
        nc.scalar.dma_start(lv(O_sh2), im(r0 + 3))  # rows r0+2p+3

    # stage-1 temps live in halves of stage-2 slots that are written later
    qpair = pm.tile([P, FP2], BF16, tag="Pmx", name="qpair")
    tpair = pm.tile([P, FP2], BF16, tag="Qmx", name="tpair")
    MN2 = pm.tile([P, FP2], BF16, tag="fMN", name="fMN")
    MD2 = pm.tile([P, FP2], BF16, tag="fMD", name="fMD")
    MX2 = pm.tile([P, FP2], BF16, tag="fMX", name="fMX")

    tt = nc.vector.tensor_tensor

    def stage1(ia, ib):
        a, b = ia * PW, ib * PW
        qv = lambda h: qpair[:][:, h * FP + a : h * FP + b]
        tv = lambda h: tpair[:][:, h * FP + a : h * FP + b]
        f = lambda T, h: T[:][:, h * FP + a : h * FP + b]
        sv = lambda T: T[:][:, a:b]
        qmn, qmx = qv(0), qv(1)
        # shared pair = (O, E_sh) = rows (2p+1, 2p+2)
        tt(qmn, sv(O), sv(E_sh), op=MIN)
        tt(qmx, sv(O), sv(E_sh), op=MAX)
        # field completion order MN, MX, MD matches the ACT copy order in
        # _stage2_fused so no s1 copy ever stalls the DVE.
        # odd output rows r0+2p+1: pair + E; even rows: pair + O_sh2
        tt(f(MN2, 0), qmn, sv(E), op=MIN)
        tt(f(MN2, 1), qmn, sv(O_sh2), op=MIN)
        tt(f(MX2, 0), qmx, sv(E), op=MAX)
        tt(f(MX2, 1), qmx, sv(O_sh2), op=MAX)
        tt(tv(0), qmx, sv(E), op=MIN)
        tt(f(MD2, 0), qmn, tv(0), op=MAX)
        tt(tv(1), qmx, sv(O_sh2), op=MIN)
        tt(f(MD2, 1), qmn, tv(1), op=MAX)

    if first:
        # ramp: small first chunk so the DVE starts early, one big second
        # chunk to keep the per-op overhead added by splitting low
        for ia, ib in ((0, 2), (2, GIMG)):
            loads(ia, ib)
            stage1(ia, ib)
    else:
        loads(0, GIMG)
        stage1(0, GIMG)

    OUT = pio.tile([P, FP2], BF16, tag="OUT", name="OUT")
    out_img = lambda r_lo: oh[r_lo : min(r_lo + 2 * P, H) : 2, i0 : i0 + GIMG, :]
    hi = GIMG // 2
    osv = _seg(OUT, P, NSEG)
    if last:
        # the end-of-kernel critical path is (last stores + HBM write ack):
        # stream each quarter of the output out the moment its final op
        # completes, rotating across the three DGE queues (the 16 SDMA
        # engines are shared, so this is about starting early, not BW)
        # first and last chunks on HWDGE (fast ~0.6us first-byte; both
        # queues are empty by now): first starts the drain ASAP, last is
        # the end-of-kernel critical path
        queues = [nc.scalar, nc.gpsimd, nc.gpsimd, nc.sync]

        def store_chunk(s0, s1_):
            r_lo = (r0 + 1) if s0 < GIMG else (r0 + 2)
            ia, ib = s0 % GIMG, (s1_ - 1) % GIMG + 1
            dst = oh[
                r_lo : min(r_lo + 2 * P, H) : 2, i0 + ia : i0 + ib, :
            ]
            queues[0].dma_start(dst, osv[:, s0:s1_, 1:513])
            queues.pop(0)

        _stage2_fused(nc, pm, MN2, MD2, MX2, OUT, stores=store_chunk)
    else:
        _stage2_fused(nc, pm, MN2, MD2, MX2, OUT)
        # stores on the SWDGE queue so they never block later blocks' loads
        nc.gpsimd.dma_start(out_img(r0 + 1), osv[:, 0:GIMG, 1:513])
        nc.gpsimd.dma_start(out_img(r0 + 2), osv[:, GIMG:NSEG, 1:513])


EP = 96   # edge partitions: p = chunk*24 + (edge*12 + img), 4 col-chunks
EW = 130  # edge grid width; position j of chunk c holds col 128c-1+j


def _edge_loads(nc, pio, xi):
    """Loads for image rows 0 and 511, reshaped to [96, 130]: each of the
    24 (edge,img) rows is split into 4 column-chunks of 128 with a 1-col
    halo on each side; the image-boundary halo positions are zeroed, so
    the pad columns fold into the grid and the edge compute needs NO
    boundary special-casing. Issued up front (tiny) so the end-of-kernel
    edge compute never waits on DMA."""
    R0 = pio.tile([EP, EW], BF16, tag="R0", name="R0")
    R1 = pio.tile([EP, EW], BF16, tag="R1", name="R1")
    for T in (R0, R1):
        # zero both halo columns on all partitions (the verifier rejects
        # partition ranges not starting at 0); loads then overwrite the
        # non-pad ones, leaving zeros only at chunk 0 col -1 / chunk 3
        # col 512
        nc.gpsimd.memset(T[:][0:EP, 0:1], 0.0)
        nc.gpsimd.memset(T[:][0:EP, 129:130], 0.0)
    for T, rows in ((R0, (0, H - 1)), (R1, (1, H - 2))):
        for k, r in enumerate(rows):  # k=0: slots 0..11, k=1: slots 12..23
            q = nc.sync if k == 0 else nc.scalar
            for c in range(4):
                p0 = 24 * c + 12 * k
                a, b = (1, 130) if c == 0 else (0, 129) if c == 3 else (0, 130)
                cols = slice(128 * c - 1 + a, 128 * c - 1 + b)
                q.dma_start(T[:][p0 : p0 + 12, a:b], xi[:, r, cols])
    return R0, R1


def _edge_compute(nc, pio, pm, oi, R0, R1):
    """Rows 0 and 511 (windows contain the zero pad row). Runs last, in
    the shadow of the final block's output stores."""
    NI = EW - 2  # 128

    def t2(tag):
        return pm.tile([EP, EW], BF16, tag=f"e{tag}", name=f"e{tag}")

    rmn, rmx = t2("rmn"), t2("rmx")
    tt = nc.vector.tensor_tensor
    tt(rmn[:], R0[:], R1[:], op=MIN)
    tt(rmx[:], R0[:], R1[:], op=MAX)

    # vertical sort3 with the zero pad row: min/max vs 0, med=max(mn,min(mx,0))
    MN0, MD0, MX0 = t2("MN"), t2("MD"), t2("MX")
    nc.vector.tensor_scalar_min(MN0[:], rmn[:], 0.0)
    nc.vector.tensor_scalar_max(MX0[:], rmx[:], 0.0)
    nc.vector.scalar_tensor_tensor(MD0[:], rmx[:], 0.0, rmn[:], op0=MIN, op1=MAX)

    # shifts on the DVE itself (2x_2P copies, ~0.2us each): the edge pass
    # must not touch the scalar engine, whose queue holds the LAST block's
    # loads right behind these instructions
    s1 = {}
    for name, F in (("MN", MN0), ("MD", MD0), ("MX", MX0)):
        T = t2(f"s1{name}")
        nc.vector.tensor_copy(T[:][0:EP, 0 : EW - 1], F[:][0:EP, 1:EW])
        s1[name] = T

    fl = lambda T, a, b: T[:][0:EP, a:b]
    Pmn, Pmx, Qmn, Qmx = t2("Pmn"), t2("Pmx"), t2("Qmn"), t2("Qmx")
    tmd, Rmn, Rmd, Rmx = t2("tmd"), t2("Rmn"), t2("Rmd"), t2("Rmx")
    MN1, MX1, TF = t2("MN1"), t2("MX1"), t2("TF")
    OUT0 = pio.tile([EP, EW], BF16, tag="OUT0", name="OUT0")

    tt(fl(Pmn, 0, NI), fl(MN0, 0, NI), fl(s1["MN"], 0, NI), op=MAX)
    tt(fl(Qmn, 0, NI), fl(MD0, 0, NI), fl(s1["MD"], 0, NI), op=MIN)
    tt(fl(Qmx, 0, NI), fl(MD0, 0, NI), fl(s1["MD"], 0, NI), op=MAX)
    tt(fl(Pmx, 0, NI), fl(MX0, 0, NI), fl(s1["MX"], 0, NI), op=MIN)
    tt(fl(Rmn, 0, NI), fl(Pmn, 0, NI), fl(MN0, 2, EW), op=MAX)
    tt(fl(tmd, 0, NI), fl(Qmx, 0, NI), fl(MD0, 2, EW), op=MIN)
    tt(fl(Rmd, 0, NI), fl(Qmn, 0, NI), fl(tmd, 0, NI), op=MAX)
    tt(fl(Rmx, 0, NI), fl(Pmx, 0, NI), fl(MX0, 2, EW), op=MIN)
    tt(fl(MN1, 0, NI), fl(Rmn, 0, NI), fl(Rmd, 0, NI), op=MIN)
    tt(fl(MX1, 0, NI), fl(Rmn, 0, NI), fl(Rmd, 0, NI), op=MAX)
    tt(fl(TF, 0, NI), fl(MX1, 0, NI), fl(Rmx, 0, NI), op=MIN)
    tt(fl(OUT0, 2, EW), fl(MN1, 0, NI), fl(TF, 0, NI), op=MAX)

    # SWDGE is empty by now (its last work was the mid-kernel stores), so
    # these tiny final stores' completion overlaps the HWDGE stores' ack
    for k, r in enumerate((0, H - 1)):
        for c in range(4):
            p0 = 24 * c + 12 * k
            nc.gpsimd.dma_start(
                oi[:, r, 128 * c : 128 * c + 128],
                OUT0[:][p0 : p0 + 12, 2:EW],
            )


def build_program():
    nc = bacc.Bacc(
        "TRN2", target_bir_lowering=False, debug=False, num_devices=N_CORES
    )
    x_d = nc.dram_tensor("x", [B_PER, C, H, W], BF16, kind="ExternalInput").ap()
    o_d = nc.dram_tensor("out", [B_PER, C, H, W], BF16, kind="ExternalOutput").ap()
    xh = x_d.rearrange("b c h w -> h (b c) w")  # [512, 12, 512]
    oh = o_d.rearrange("b c h w -> h (b c) w")
    xi = x_d.rearrange("b c h w -> (b c) h w")  # [12, 512, 512]
    oi = o_d.rearrange("b c h w -> (b c) h w")

    with tile.TileContext(nc) as tc:
        with (
            tc.tile_pool(name="io", bufs=1) as pio,
            tc.tile_pool(name="mid", bufs=1) as pm,
        ):
            nb = 2 * (NIMG // GIMG)
            _block(nc, pio, pm, xh, oh, 0, 0, first=True)
            R0, R1 = _edge_loads(nc, pio, xi)
            for i in range(1, nb - 1):
                _block(nc, pio, pm, xh, oh, i // 2, i % 2)
            # edge pass second-to-last: its stores drain under the final
            # block's compute, leaving only that block's (split) stores +
            # HBM write-ack on the end-of-kernel critical path
            _edge_compute(nc, pio, pm, oi, R0, R1)
            _block(nc, pio, pm, xh, oh, (nb - 1) // 2, (nb - 1) % 2,
                   last=True)
    nc.compile()
    return nc


def _get_program():
    global _PROGRAM
    if _PROGRAM is None:
        _PROGRAM = build_program()
    return _PROGRAM


def make_in_maps(x: np.ndarray):
    xb = np.ascontiguousarray(x).astype(ml_dtypes.bfloat16)
    return [{"x": xb[k * B_PER : (k + 1) * B_PER]} for k in range(N_CORES)]


def kernel(**inputs) -> np.ndarray:
    x = np.asarray(inputs["x"], dtype=np.float32)
    assert x.shape == (B, C, H, W), x.shape
    nc = _get_program()
    res = bass_utils.run_bass_kernel_spmd(
        nc, make_in_maps(x), core_ids=list(range(N_CORES))
    )
    out = np.concatenate(
        [np.asarray(res.results[k]["out"]) for k in range(N_CORES)], axis=0
    )
    return out.astype(np.float32)
